# revision 10
# baseline (speedup 1.0000x reference)
"""Multi-head causal attention (B=4, S=2048, D=1024, H=16) on 8 Trainium2 cores.

Sharding: core c handles batch c//2 and heads [8*(c%2), 8*(c%2)+8).
Each core computes a partial output (its 8 heads' contribution to all 2048
rows of its batch); the host sums the two partials per batch. No collectives.

Pipeline (fp16 operands, fp8e4m3 hi/lo DoubleRow matmuls, f32 PSUM):
  x    arrives pre-transposed and hi/lo fp8-split from the host (xhl_in);
         weights arrive fp8 hi/lo, pre-scaled by 64 so the lo residuals
         stay out of e4m3 subnormals, packed in DoubleRow slot layout
  Q/K/V projections: DoubleRow fp8 matmuls at 0.5 cycles/row,
         w^T x ~= wh^T(xh+xl) + wl^T xh (relative error ~0.1%); the
         un-duplicated hi weights enter via 0-stride broadcast APs
  scores^T = Kt^T Qt per (pair, 512-q tile, 128-k tile), band-narrowed
  P^T  = exp(scores/8) on ACT, causal diag masked in place by one strided
         gpsimd affine_select per k-tile (both heads at once)
  PV flipped: attn[q,65] += P^T[k,qsub]^T V[k,65] (psum bank = 4 chains
         behind one start=True region-zero; the two banks are software-
         pipelined 2 kt-iterations apart so softmax tails overlap PV)
  column 64 of V holds 1/32 -> per-partition softmax sums -> DVE
         reciprocal + tensor_scalar normalize (at_sb holds 32*attn);
         attn_n -> at_sb via one DMA-crossbar transpose per half-qt,
         then Pool casts the fp8 hi/lo copy used by the out-projection
  out-proj (at^T w_out) in fp8 DoubleRow, interleaved into pair-3's
         attention as filler; partials summed on the host per batch

Q/K projections of pair p+1 are interleaved as filler into the attention
of pair p (fractionally paced); the out projection is the filler of
pair 3. Startup DMAs are few and large (HWDGE issue overhead dominates
small transfers) and ordered by first consumer.
"""
import sys
sys.path.insert(0, '/opt/trn_rl_repo')

import numpy as np

B, S, DM, H, HS = 4, 2048, 1024, 16, 64
HPC = 8      # heads per core
NPAIR = 4    # head pairs per core
NQT = 4      # q tiles of 512
SCALE = 1.0 / np.sqrt(HS)
WSCALE = 64.0  # fp8 hi/lo weight pre-scale (keeps residuals out of subnormals)
ASCALE = 32.0  # fp8 hi/lo pre-scale for the attention output path

_CACHE = {}


def _build():
    import concourse.bass as bass
    import concourse.bacc as bacc
    import concourse.mybir as mybir
    from concourse.tile import TileContext

    f32 = mybir.dt.float32
    f16 = mybir.dt.float16
    f8 = mybir.dt.float8e4
    DR = mybir.MatmulPerfMode.DoubleRow

    nc = bacc.Bacc()
    # x pre-transposed and hi/lo fp8-split host-side:
    # col = s*16384 + ki*2048 + seq, row = dm%128 (ki = dm//128)
    xhl_in = nc.dram_tensor("xhl_in", [128, 2 * 8 * S], f8, kind="ExternalInput")
    woh_in = nc.dram_tensor("woh_in", [128, 4 * 1024], f8, kind="ExternalInput")
    wol_in = nc.dram_tensor("wol_in", [128, 4 * 1024], f8, kind="ExternalInput")
    # fp8 hi/lo Q/K weights, one [qh|ql|kh|kl] block per pair so each
    # pair preload is a single DMA
    wqk_in = nc.dram_tensor("wqk_in", [128, 4 * 4096], f8, kind="ExternalInput")
    wvh_in = nc.dram_tensor("wvh_in", [128, 8 * 512], f8, kind="ExternalInput")
    wvl_in = nc.dram_tensor("wvl_in", [128, 4 * 1024], f8, kind="ExternalInput")
    out_ex = nc.dram_tensor("out_partial", [S, DM], f16, kind="ExternalOutput")

    # band plan: for band k-tile with offset dlt, compute cols [c0, 512)
    BAND = {0: 0, 128: 128, 256: 256, 384: 384}

    with TileContext(nc) as tc:
        with tc.tile_pool(name="persist", bufs=1) as persist, \
             tc.tile_pool(name="prj", bufs=4) as prj, \
             tc.tile_pool(name="attp", bufs=2) as attp, \
             tc.tile_pool(name="osbp", bufs=2) as osbp, \
             tc.tile_pool(name="psp", bufs=2, space="PSUM") as psp:

            qt_sb = persist.tile([128, NPAIR * S], f16)          # 2 MB
            kt_sb = persist.tile([128, NPAIR * S], f16)          # 2 MB
            v_sb = persist.tile([128, 16 * HPC * 65], f16)       # 2.1 MB
            at_sb = persist.tile([128, NPAIR * S], f16)          # 2 MB
            ath_sb = persist.tile([128, NPAIR * S], f8)          # 1 MB
            atl_sb = persist.tile([128, NPAIR * S], f8)          # 1 MB
            woh_sb = persist.tile([128, 4 * 1024], f8)           # 0.5 MB
            wol_sb = persist.tile([128, 4 * 1024], f8)           # 0.5 MB
            wvh_sb = persist.tile([128, 8 * 512], f8)            # 0.5 MB
            wvl_sb = persist.tile([128, 4 * 1024], f8)           # 0.5 MB
            wqk_sb = persist.tile([128, 4 * 4096], f8)           # 2 MB
            # xh/xl fp8: col = s*16384 + ki*2048 + seq
            xhl = persist.tile([128, 2 * 8 * S], f8)             # 4 MB
            xhl_s = xhl.rearrange("p (s r) -> p s r", s=2)
            xhl_a = xhl.rearrange("p (a b) -> p a b", b=S)

            # ---------- startup DMAs, ordered by first consumer ----------
            with nc.named_scope("ph1_load"):
                # few large transfers: HWDGE issue overhead dominates many
                # small ones; order tracks the V-proj ki consumption
                def c0_piece(s, k0, k1):
                    view = (lambda t: t.rearrange("p (s2 k b) -> p s2 k b",
                                                  s2=2, b=S))
                    nc.sync.dma_start(
                        view(xhl)[:, s, k0:k1, 0:512],
                        view(xhl_in)[:, s, k0:k1, 0:512])
                # small lead slices let V-proj pos0 start ~3us sooner;
                # the remainder stays in large transfers
                nc.sync.dma_start(wvh_sb[:, 0:1024], wvh_in[:, 0:1024])
                c0_piece(0, 0, 2)
                c0_piece(1, 0, 2)
                nc.sync.dma_start(wvh_sb[:, 1024:4096], wvh_in[:, 1024:4096])
                c0_piece(0, 2, 4)
                c0_piece(1, 2, 4)
                nc.sync.dma_start(wvl_sb[:], wvl_in[:])
                c0_piece(0, 4, 8)
                c0_piece(1, 4, 8)
                nc.sync.dma_start(wqk_sb[:, 0:4096], wqk_in[:, 0:4096])
                nc.sync.dma_start(wqk_sb[:, 4096:8192], wqk_in[:, 4096:8192])

            # "ones" columns of v_sb at 1/WSCALE: sums come out pre-divided,
            # so the reciprocal yields WSCALE/sum and at_sb holds WSCALE*attn
            # (keeps the fp8 hi/lo split of at_sb out of subnormals)
            for pos in range(16):
                nc.vector.memset(
                    v_sb[:, pos * 520:(pos + 1) * 520]
                    .rearrange("p (h c) -> p h c", c=65)[:, :, 64:65],
                    1.0 / ASCALE)

            # ---------- projection emitters ----------
            def vproj8_thunks(pos):
                """fp8 DoubleRow V projection of seq tile pos as thunks."""
                thunks = []
                state = {}

                def alloc():
                    state["psv"] = psp.tile([128, 512], f32, tag="projacc",
                                            name="psv")
                thunks.append(alloc)
                for ki in range(8):
                    def mm(ki=ki):
                        nc.tensor.matmul(
                            state["psv"][:],
                            xhl_s[:, :, ki * 2048 + pos * 128:
                                  ki * 2048 + pos * 128 + 128],
                            wvh_sb[:, ki * 512:(ki + 1) * 512]
                            .rearrange("p (o c) -> p o c", o=1)
                            .broadcast_to([128, 2, 512]),
                            start=(ki == 0), stop=False, perf_mode=DR)
                    thunks.append(mm)
                for kd in range(4):
                    def mm(kd=kd):
                        nc.tensor.matmul(
                            state["psv"][:],
                            xhl_a[:, 2 * kd:2 * kd + 2,
                                  pos * 128:pos * 128 + 128],
                            wvl_sb.rearrange("p (d s c) -> p d s c",
                                             s=2, c=512)[:, kd],
                            start=False, stop=(kd == 3), perf_mode=DR)
                    thunks.append(mm)

                def cp():
                    with nc.allow_low_precision(reason="f16 V"):
                        nc.vector.tensor_scalar_mul(
                            v_sb[:, pos * 520: (pos + 1) * 520]
                            .rearrange("p (h c) -> p h c", c=65)[:, :, 0:64],
                            state["psv"].rearrange("p (h c) -> p h c", c=64),
                            1.0 / WSCALE)
                thunks.append(cp)
                return thunks

            def qkproj_thunks(p, qcs=range(4), state=None, preload=True):
                """fp8 DoubleRow Q/K projection of pair p over q-chunks qcs."""
                thunks = []
                if state is None:
                    state = {}
                if preload:
                    def pre(p=p):
                        wqk = wqk_sb[:, p * 4096:(p + 1) * 4096]
                        state["q"] = (wqk[:, 0:1024], wqk[:, 1024:2048])
                        state["k"] = (wqk[:, 2048:3072], wqk[:, 3072:4096])
                    thunks.append(pre)

                for key, dest in (("q", qt_sb), ("k", kt_sb)):
                    for qc in qcs:
                        for ki in range(8):
                            def mm(key=key, qc=qc, ki=ki):
                                if ki == 0:
                                    state["acc"] = psp.tile(
                                        [128, 512], f32, tag="projacc",
                                        name="projacc")
                                nc.tensor.matmul(
                                    state["acc"][:],
                                    state[key][0][:, ki * 128:(ki + 1) * 128]
                                    .rearrange("p (o m) -> p o m", o=1)
                                    .broadcast_to([128, 2, 128]),
                                    xhl_s[:, :, ki * 2048 + qc * 512:
                                          ki * 2048 + (qc + 1) * 512],
                                    start=(ki == 0), stop=False, perf_mode=DR)
                            thunks.append(mm)
                        for kd in range(4):
                            def mm(key=key, qc=qc, kd=kd):
                                nc.tensor.matmul(
                                    state["acc"][:],
                                    state[key][1].rearrange(
                                        "p (d s m) -> p d s m",
                                        s=2, m=128)[:, kd],
                                    xhl_a[:, 2 * kd:2 * kd + 2,
                                          qc * 512:(qc + 1) * 512],
                                    start=False, stop=(kd == 3), perf_mode=DR)
                            thunks.append(mm)

                        def cp(dest=dest, qc=qc, p=p):
                            with nc.allow_low_precision(reason="f16 qkv"):
                                nc.vector.tensor_scalar_mul(
                                    dest[:, p * S + qc * 512:
                                         p * S + (qc + 1) * 512],
                                    state["acc"][:], 1.0 / WSCALE)
                        thunks.append(cp)
                return thunks

            def outproj_thunks(qt, js=range(4)):
                """Out-projection of q16 tiles qt*4+j for j in js."""
                thunks = []
                state = {}
                for j in js:
                    q16 = qt * 4 + j

                    def alloc(q16=q16):
                        state["pso"] = [
                            psp.tile([128, 512], f32, tag="projacc",
                                     name=f"pso{fc}") for fc in range(2)]
                        state["osb"] = osbp.tile([128, 1024], f16, tag="osb",
                                                 bufs=4, name="osb")
                    thunks.append(alloc)

                    def at2(src, pp, q16):
                        """[128, 2, 128] DoubleRow slots over pairs pp, pp+1."""
                        return src.rearrange("p (e c) -> p e c", e=4) \
                            [:, pp:pp + 2, q16 * 128:(q16 + 1) * 128]

                    for term, src, wsb in (("hh", ath_sb, woh_sb),
                                           ("lh", atl_sb, woh_sb),
                                           ("hl", ath_sb, wol_sb)):
                        for pp in (0, 2):
                            for fc in range(2):
                                def mm(term=term, src=src, wsb=wsb,
                                       pp=pp, fc=fc, q16=q16):
                                    nc.tensor.matmul(
                                        state["pso"][fc][:],
                                        at2(src, pp, q16),
                                        wsb.rearrange(
                                            "p (e c) -> p e c",
                                            e=4)[:, pp:pp + 2,
                                                 fc * 512:(fc + 1) * 512],
                                        start=(term == "hh" and pp == 0),
                                        stop=(term == "hl" and pp == 2),
                                        perf_mode=DR)
                                thunks.append(mm)

                    def cp(q16=q16):
                        for fc in range(2):
                            with nc.allow_low_precision(reason="descale"):
                                nc.vector.tensor_scalar_mul(
                                    state["osb"][:, fc * 512:(fc + 1) * 512],
                                    state["pso"][fc][:],
                                    1.0 / (ASCALE * WSCALE))
                        # SWDGE store off the SP queue so it never waits
                        # behind the at_sb transposes' SEQ-held waits
                        nc.gpsimd.dma_start(
                            out_ex[q16 * 128:(q16 + 1) * 128, :], state["osb"][:])
                    thunks.append(cp)
                return thunks

            # ---------- bootstrap: V(0-3) + pair-0 qc0 ----------
            p0_state = {}
            with nc.named_scope("ph2_boot"):
                boot = []
                for pos in range(4):
                    boot += vproj8_thunks(pos)
                for t in qkproj_thunks(0, qcs=[0], state=p0_state)[:1]:
                    t()  # pair-0 weight DMAs first (ahead of x chunks 1-3)
                # seq chunks 1-3 of xh/xl (strided 3D copies), with the
                # remaining weight blocks interleaved by first consumer
                for c in range(1, 4):
                    for s in range(2):
                        nc.sync.dma_start(
                            xhl_s[:, s].rearrange("p (k b) -> p k b", k=8)
                            [:, :, c * 512:(c + 1) * 512],
                            xhl_in.rearrange("p (s k b) -> p s k b",
                                             s=2, b=S)[:, s, :,
                                                       c * 512:(c + 1) * 512])
                    if c == 1:
                        nc.sync.dma_start(wqk_sb[:, 8192:12288],
                                          wqk_in[:, 8192:12288])
                    elif c == 2:
                        nc.sync.dma_start(wqk_sb[:, 12288:16384],
                                          wqk_in[:, 12288:16384])
                    else:
                        nc.sync.dma_start(woh_sb[:], woh_in[:])
                        nc.sync.dma_start(wol_sb[:], wol_in[:])
                for t in boot:
                    t()
                for t in qkproj_thunks(0, qcs=[0], state=p0_state,
                                       preload=False):
                    t()

            # persistent PV accumulators: 4 chains x 65 cols per bank tile.
            # The first matmul of each qt (start=True) zeroes the whole psum
            # bank region, so all four chains start from 0 with no memset.
            pvq = [psp.tile([128, 512], f32, tag=f"pvq{i}", name=f"pvq{i}",
                            bufs=1) for i in range(2)]

            with nc.named_scope("ph3_attn"):
                for p in range(NPAIR):
                    if p + 1 < NPAIR:
                        filler = qkproj_thunks(p + 1)
                    else:
                        filler = []  # pair 3: out-proj of prior qt is filler
                    fi = 0
                    prev_qt = None
                    for qt in range(NQT):
                        if p == 0 and qt > 0:
                            # feed pair-0's own pipeline: next V tiles + qc
                            for t in vproj8_thunks(4 * qt):
                                t()
                            for t in vproj8_thunks(4 * qt + 1):
                                t()
                            for t in vproj8_thunks(4 * qt + 2):
                                t()
                            for t in vproj8_thunks(4 * qt + 3):
                                t()
                            for t in qkproj_thunks(0, qcs=[qt],
                                                   state=p0_state,
                                                   preload=False):
                                t()
                        if p == NPAIR - 1:
                            if prev_qt is not None:
                                filler = filler[fi:] + outproj_thunks(prev_qt)
                            else:
                                filler = []
                            fi = 0
                        prev_qt = qt
                        qs = qt * 512
                        nkt = qs // 128 + 4
                        # pair 3's filler (out-proj) waits on the previous
                        # qt's at_sb transpose: hold it back 2 slots
                        skip_slots = 2 if p == NPAIR - 1 else 0
                        n_slots = nkt + 4 - skip_slots
                        rate = max(0, len(filler) - fi) / n_slots
                        facc = 0.0
                        fired = 0
                        rc = attp.tile([128, 8], f32, tag="rc", bufs=6,
                                       name="rc")
                        an = attp.tile([128, 512], f16, tag="an", bufs=6,
                                       name="an")

                        def softmax_tail(i):
                            """reciprocal + normalize of pvq[i] (qsubs 2i,
                            2i+1) into the an tile, then transpose out its
                            half of at_sb so consumers start early."""
                            nc.vector.reciprocal(
                                rc.rearrange("p (i g o) -> p i g o",
                                             i=2, o=1)[:, i],
                                pvq[i][:, 0:260]
                                .rearrange("p (g c) -> p g c", c=65)[:, :, 64:65])
                            for qs2 in range(2):
                                qsub = 2 * i + qs2
                                base = i * 4 + qs2 * 2
                                with nc.allow_low_precision(reason="f16 attn"):
                                    # both heads in one op: per-(partition,h)
                                    # reciprocal enters as a 0-stride bcast
                                    nc.vector.tensor_mul(
                                        an[:, qsub * 128:(qsub + 1) * 128]
                                        .rearrange("p (s d) -> p s d", s=2),
                                        pvq[i][:, qs2 * 130: qs2 * 130 + 130]
                                        .rearrange("p (s c) -> p s c",
                                                   c=65)[:, :, 0:64],
                                        rc[:, base:base + 2]
                                        .rearrange("p (s o) -> p s o", o=1)
                                        .broadcast_to([128, 2, 64]))
                            cols = slice(p * S + qs + i * 256,
                                         p * S + qs + i * 256 + 256)
                            nc.sync.dma_start(
                                at_sb[:, cols].rearrange("d (e j) -> d e j",
                                                         e=2),
                                an[:, i * 256:(i + 1) * 256], transpose=True)
                            # fp8 hi/lo split for the out-projection (Pool)
                            with nc.allow_low_precision(reason="fp8 at"):
                                nc.gpsimd.tensor_copy(ath_sb[:, cols],
                                                      at_sb[:, cols])
                                nc.gpsimd.tensor_sub(atl_sb[:, cols],
                                                     at_sb[:, cols],
                                                     ath_sb[:, cols])

                        def pv_bank(i, ktm):
                            """PV chains of bank i (qsubs 2i, 2i+1) for ktm."""
                            ptile, c0 = ptiles[ktm]
                            for h in range(2):
                                for qsub in range(max(2 * i, c0 // 128),
                                                  2 * i + 2):
                                    po = (qsub % 2) * 130 + h * 65
                                    nc.tensor.matmul(
                                        pvq[i][:, po:po + 65],
                                        ptile[:, h * 512 + qsub * 128:
                                              h * 512 + (qsub + 1) * 128],
                                        v_sb[:, ktm * 520 + (2 * p + h) * 65:
                                             ktm * 520 + (2 * p + h) * 65 + 65],
                                        start=(ktm == 0 and h == 0
                                               and qsub == 2 * i),
                                        stop=(h == 1 and qsub == 2 * i + 1
                                              and ktm == qs // 128 + qsub))

                        ptiles = {}
                        for kt in range(nkt + 4):
                            if kt < nkt:
                                dlt = kt * 128 - qs
                                c0 = BAND[dlt] if dlt >= 0 else 0
                                sc = psp.tile([128, 1024], f32, tag="sc",
                                              name="sc")
                                for h in range(2):
                                    nc.tensor.matmul(
                                        sc[:, h * 512 + c0: (h + 1) * 512],
                                        kt_sb[h * 64:(h + 1) * 64,
                                              p * S + kt * 128: p * S + (kt + 1) * 128],
                                        qt_sb[h * 64:(h + 1) * 64,
                                              p * S + qs + c0: p * S + qs + 512],
                                        start=True, stop=True)
                                ptile = attp.tile([128, 1024], f16, tag="pt",
                                                  bufs=8, name="pt")
                                sc_v = sc.rearrange("k (h q) -> k h q", h=2)[:, :, c0:512]
                                pt_v = ptile.rearrange("k (h q) -> k h q", h=2)[:, :, c0:512]
                                with nc.allow_low_precision(reason="f16 probs"):
                                    nc.scalar.activation(
                                        pt_v, sc_v, mybir.ActivationFunctionType.Exp,
                                        bias=0.0, scale=float(SCALE))
                                if dlt >= 0:
                                    blk = ptile.rearrange(
                                        "k (h q) -> k h q",
                                        h=2)[:, :, dlt:dlt + 128]
                                    nc.gpsimd.affine_select(
                                        out=blk, in_=blk,
                                        compare_op=mybir.AluOpType.is_ge,
                                        fill=0.0, base=0,
                                        pattern=[[0, 2], [1, 128]],
                                        channel_multiplier=-1)
                                ptiles[kt] = (ptile, c0)
                            if 2 <= kt < nkt + 2:   # bank 0: PV for kt-2
                                ktm = kt - 2
                                if ptiles[ktm][1] // 128 < 2:
                                    pv_bank(0, ktm)
                                if ktm == nkt - 3:
                                    softmax_tail(0)  # qsubs 0,1 complete
                            if kt >= 4:             # bank 1: PV for kt-4
                                ktm = kt - 4
                                pv_bank(1, ktm)
                                ptiles.pop(ktm)
                            if kt >= skip_slots:
                                facc += rate
                                while fired < int(facc) and fi < len(filler):
                                    filler[fi]()
                                    fi += 1
                                    fired += 1
                        # tail: qsubs 2,3 normalize + transpose
                        softmax_tail(1)
                    # flush remaining filler at end of pair
                    for t in filler[fi:]:
                        t()
                    fi = len(filler)

            # ---------- phase 4: final out projection chunk ----------
            with nc.named_scope("ph4_outproj"):
                for t in outproj_thunks(NQT - 1):
                    t()

    nc.compile()
    return nc


def _get_program():
    if "nc" not in _CACHE:
        _CACHE["nc"] = _build()
    return _CACHE["nc"]


def _split8(w):
    import ml_dtypes
    f8 = ml_dtypes.float8_e4m3
    h = w.astype(f8)
    l = (w - h.astype(np.float32)).astype(f8)
    return h, l


def _make_in_maps(x, w_qkv, w_out):
    import ml_dtypes
    f8np = ml_dtypes.float8_e4m3
    x = np.asarray(x, dtype=np.float32).astype(np.float16)
    w_qkv = np.asarray(w_qkv, dtype=np.float32)
    w_out = np.asarray(w_out, dtype=np.float32)

    w_q, w_k, w_v = w_qkv[:, 0:DM], w_qkv[:, DM:2 * DM], w_qkv[:, 2 * DM:3 * DM]

    def pack_x(xb16):
        """[2048, 1024] fp16 -> [128, 2*8*2048] fp8 hi/lo, transposed."""
        xh = xb16.astype(f8np)
        xl = (xb16.astype(np.float32) - xh.astype(np.float32)).astype(f8np)

        def tr(part):
            return part.T.reshape(8, 128, S).transpose(1, 0, 2).reshape(128, 8 * S)
        return np.ascontiguousarray(
            np.concatenate([tr(xh), tr(xl)], axis=1))

    xhl_packs = [pack_x(x[b]) for b in range(B)]

    def pack_qk(w_core):
        """[1024, 512] -> hi [128, 4*2048], lo [128, 4*1024] DoubleRow packs."""
        w_core = w_core * WSCALE
        his, los = [], []
        for p in range(NPAIR):
            h, l = _split8(w_core[:, p * 128:(p + 1) * 128])
            # hi: [r, ki, m] (DoubleRow dup comes from a 0-stride broadcast)
            hp = h.reshape(8, 128, 128).transpose(1, 0, 2)
            his.append(np.ascontiguousarray(hp.reshape(128, 1024)))
            # lo: [r, kd, s, m] with s the ki sub-chunk
            lp = l.reshape(4, 2, 128, 128).transpose(2, 0, 1, 3)
            los.append(lp.reshape(128, 1024))
        return np.concatenate(his, 1), np.concatenate(los, 1)

    def pack_v(w_core):
        """[1024, 512] -> hi [128, 8*512], lo [128, 4*1024]."""
        h, l = _split8(w_core * WSCALE)
        hp = h.reshape(8, 128, 512).transpose(1, 0, 2)
        lp = l.reshape(4, 2, 128, 512).transpose(2, 0, 1, 3)
        return (np.ascontiguousarray(hp.reshape(128, 4096)),
                np.ascontiguousarray(lp.reshape(128, 4096)))

    in_maps = []
    for c in range(8):
        b, s = c // 2, c % 2
        cols = slice(s * HPC * HS, (s + 1) * HPC * HS)
        wqh, wql = pack_qk(w_q[:, cols])
        wkh, wkl = pack_qk(w_k[:, cols])
        # one [qh|ql|kh|kl] block per pair
        wqk = np.concatenate(
            [np.concatenate([wqh[:, p * 1024:(p + 1) * 1024],
                             wql[:, p * 1024:(p + 1) * 1024],
                             wkh[:, p * 1024:(p + 1) * 1024],
                             wkl[:, p * 1024:(p + 1) * 1024]], axis=1)
             for p in range(NPAIR)], axis=1)
        wvh, wvl = pack_v(w_v[:, cols])
        # wo: [512, 1024] scaled, fp8 hi/lo, [r, pair, c] layout
        woh8, wol8 = _split8(w_out[cols, :] * WSCALE)
        woh = np.ascontiguousarray(
            woh8.reshape(4, 128, 1024).transpose(1, 0, 2).reshape(128, 4096))
        wol = np.ascontiguousarray(
            wol8.reshape(4, 128, 1024).transpose(1, 0, 2).reshape(128, 4096))
        in_maps.append({
            "xhl_in": xhl_packs[b],
            "woh_in": woh, "wol_in": wol,
            "wqk_in": np.ascontiguousarray(wqk),
            "wvh_in": wvh, "wvl_in": wvl,
        })
    return in_maps


def kernel(x: np.ndarray, w_qkv: np.ndarray, w_out: np.ndarray) -> np.ndarray:
    from concourse.bass_utils import run_bass_kernel_spmd

    nc = _get_program()
    in_maps = _make_in_maps(x, w_qkv, w_out)
    res = run_bass_kernel_spmd(nc, in_maps, core_ids=list(range(8)))
    out = np.empty((B, S, DM), dtype=np.float32)
    for b in range(B):
        out[b] = (res.results[2 * b]["out_partial"].astype(np.float32)
                  + res.results[2 * b + 1]["out_partial"].astype(np.float32))
    return out



# revision 11
# speedup vs baseline: 1.0426x; 1.0426x over previous
"""Multi-head causal attention (B=4, S=2048, D=1024, H=16) on 8 Trainium2 cores.

Sharding: core c handles batch c//2 and heads [8*(c%2), 8*(c%2)+8).
Each core computes a partial output (its 8 heads' contribution to all 2048
rows of its batch); the host sums the two partials per batch. No collectives.

Pipeline (fp16 operands, fp8e4m3 hi/lo DoubleRow matmuls, f32 PSUM):
  x    arrives pre-transposed and hi/lo fp8-split from the host (xhl_in);
         weights arrive fp8 hi/lo, pre-scaled by 64 so the lo residuals
         stay out of e4m3 subnormals, packed in DoubleRow slot layout
  Q/K/V projections: DoubleRow fp8 matmuls at 0.5 cycles/row,
         w^T x ~= wh^T(xh+xl) + wl^T xh (relative error ~0.1%); the
         un-duplicated hi weights enter via 0-stride broadcast APs
  scores^T = Kt^T Qt per (pair, 512-q tile, 128-k tile), band-narrowed
  P^T  = exp(scores/8) on ACT, causal diag masked in place by one strided
         gpsimd affine_select per k-tile (both heads at once)
  PV flipped: attn[q,65] += P^T[k,qsub]^T V[k,65] (psum bank = 4 chains
         behind one start=True region-zero; the two banks are software-
         pipelined 2 kt-iterations apart so softmax tails overlap PV)
  column 64 of V holds 1/32 -> per-partition softmax sums -> DVE
         reciprocal + tensor_scalar normalize (at_sb holds 32*attn);
         attn_n -> at_sb via one DMA-crossbar transpose per half-qt,
         then Pool casts the fp8 hi/lo copy used by the out-projection
  out-proj (at^T w_out) in fp8 DoubleRow, interleaved into pair-3's
         attention as filler; partials summed on the host per batch

Q/K projections of pair p+1 are interleaved as filler into the attention
of pair p (fractionally paced); the out projection is the filler of
pair 3. Startup DMAs are few and large (HWDGE issue overhead dominates
small transfers) and ordered by first consumer.
"""
import sys
sys.path.insert(0, '/opt/trn_rl_repo')

import numpy as np

B, S, DM, H, HS = 4, 2048, 1024, 16, 64
HPC = 8      # heads per core
NPAIR = 4    # head pairs per core
NQT = 4      # q tiles of 512
SCALE = 1.0 / np.sqrt(HS)
WSCALE = 64.0  # fp8 hi/lo weight pre-scale (keeps residuals out of subnormals)
ASCALE = 32.0  # fp8 hi/lo pre-scale for the attention output path

_CACHE = {}


def _build():
    import concourse.bass as bass
    import concourse.bacc as bacc
    import concourse.mybir as mybir
    from concourse.tile import TileContext

    f32 = mybir.dt.float32
    f16 = mybir.dt.float16
    f8 = mybir.dt.float8e4
    DR = mybir.MatmulPerfMode.DoubleRow

    nc = bacc.Bacc()
    # x pre-transposed and hi/lo fp8-split host-side:
    # col = s*16384 + ki*2048 + seq, row = dm%128 (ki = dm//128)
    xhl_in = nc.dram_tensor("xhl_in", [128, 2 * 8 * S], f8, kind="ExternalInput")
    woh_in = nc.dram_tensor("woh_in", [128, 4 * 1024], f8, kind="ExternalInput")
    wol_in = nc.dram_tensor("wol_in", [128, 4 * 1024], f8, kind="ExternalInput")
    # fp8 hi/lo Q/K weights, one [qh|ql|kh|kl] block per pair so each
    # pair preload is a single DMA
    wqk_in = nc.dram_tensor("wqk_in", [128, 4 * 4096], f8, kind="ExternalInput")
    wvh_in = nc.dram_tensor("wvh_in", [128, 8 * 512], f8, kind="ExternalInput")
    wvl_in = nc.dram_tensor("wvl_in", [128, 4 * 1024], f8, kind="ExternalInput")
    out_ex = nc.dram_tensor("out_partial", [S, DM], f16, kind="ExternalOutput")

    # band plan: for band k-tile with offset dlt, compute cols [c0, 512)
    BAND = {0: 0, 128: 128, 256: 256, 384: 384}

    with TileContext(nc) as tc:
        with tc.tile_pool(name="persist", bufs=1) as persist, \
             tc.tile_pool(name="prj", bufs=4) as prj, \
             tc.tile_pool(name="attp", bufs=2) as attp, \
             tc.tile_pool(name="osbp", bufs=2) as osbp, \
             tc.tile_pool(name="psp", bufs=2, space="PSUM") as psp:

            qt_sb = persist.tile([128, NPAIR * S], f16)          # 2 MB
            kt_sb = persist.tile([128, NPAIR * S], f16)          # 2 MB
            v_sb = persist.tile([128, 16 * HPC * 65], f16)       # 2.1 MB
            at_sb = persist.tile([128, NPAIR * S], f16)          # 2 MB
            ath_sb = persist.tile([128, NPAIR * S], f8)          # 1 MB
            atl_sb = persist.tile([128, NPAIR * S], f8)          # 1 MB
            woh_sb = persist.tile([128, 4 * 1024], f8)           # 0.5 MB
            wol_sb = persist.tile([128, 4 * 1024], f8)           # 0.5 MB
            wvh_sb = persist.tile([128, 8 * 512], f8)            # 0.5 MB
            wvl_sb = persist.tile([128, 4 * 1024], f8)           # 0.5 MB
            wqk_sb = persist.tile([128, 4 * 4096], f8)           # 2 MB
            # xh/xl fp8: col = s*16384 + ki*2048 + seq
            xhl = persist.tile([128, 2 * 8 * S], f8)             # 4 MB
            xhl_s = xhl.rearrange("p (s r) -> p s r", s=2)
            xhl_a = xhl.rearrange("p (a b) -> p a b", b=S)

            # ---------- startup DMAs, ordered by first consumer ----------
            with nc.named_scope("ph1_load"):
                # few large transfers: HWDGE issue overhead dominates many
                # small ones; order tracks the V-proj ki consumption
                def c0_piece(s, k0, k1):
                    view = (lambda t: t.rearrange("p (s2 k b) -> p s2 k b",
                                                  s2=2, b=S))
                    nc.sync.dma_start(
                        view(xhl)[:, s, k0:k1, 0:512],
                        view(xhl_in)[:, s, k0:k1, 0:512])
                # small lead slices let V-proj pos0 start ~3us sooner;
                # the remainder stays in large transfers
                nc.sync.dma_start(wvh_sb[:, 0:1024], wvh_in[:, 0:1024])
                c0_piece(0, 0, 2)
                c0_piece(1, 0, 2)
                nc.sync.dma_start(wvh_sb[:, 1024:4096], wvh_in[:, 1024:4096])
                c0_piece(0, 2, 4)
                c0_piece(1, 2, 4)
                nc.sync.dma_start(wvl_sb[:], wvl_in[:])
                c0_piece(0, 4, 8)
                c0_piece(1, 4, 8)
                nc.sync.dma_start(wqk_sb[:, 0:4096], wqk_in[:, 0:4096])
                nc.sync.dma_start(wqk_sb[:, 4096:8192], wqk_in[:, 4096:8192])

            # "ones" columns of v_sb at 1/WSCALE: sums come out pre-divided,
            # so the reciprocal yields WSCALE/sum and at_sb holds WSCALE*attn
            # (keeps the fp8 hi/lo split of at_sb out of subnormals)
            for pos in range(16):
                nc.vector.memset(
                    v_sb[:, pos * 520:(pos + 1) * 520]
                    .rearrange("p (h c) -> p h c", c=65)[:, :, 64:65],
                    1.0 / ASCALE)

            # ---------- projection emitters ----------
            def vproj8_thunks(pos):
                """fp8 DoubleRow V projection of seq tile pos as thunks."""
                thunks = []
                state = {}

                def alloc():
                    state["psv"] = psp.tile([128, 512], f32, tag="projacc",
                                            name="psv")
                thunks.append(alloc)
                for ki in range(8):
                    def mm(ki=ki):
                        nc.tensor.matmul(
                            state["psv"][:],
                            xhl_s[:, :, ki * 2048 + pos * 128:
                                  ki * 2048 + pos * 128 + 128],
                            wvh_sb[:, ki * 512:(ki + 1) * 512]
                            .rearrange("p (o c) -> p o c", o=1)
                            .broadcast_to([128, 2, 512]),
                            start=(ki == 0), stop=False, perf_mode=DR)
                    thunks.append(mm)
                for kd in range(4):
                    def mm(kd=kd):
                        nc.tensor.matmul(
                            state["psv"][:],
                            xhl_a[:, 2 * kd:2 * kd + 2,
                                  pos * 128:pos * 128 + 128],
                            wvl_sb.rearrange("p (d s c) -> p d s c",
                                             s=2, c=512)[:, kd],
                            start=False, stop=(kd == 3), perf_mode=DR)
                    thunks.append(mm)

                def cp():
                    with nc.allow_low_precision(reason="f16 V"):
                        nc.vector.tensor_scalar_mul(
                            v_sb[:, pos * 520: (pos + 1) * 520]
                            .rearrange("p (h c) -> p h c", c=65)[:, :, 0:64],
                            state["psv"].rearrange("p (h c) -> p h c", c=64),
                            1.0 / WSCALE)
                thunks.append(cp)
                return thunks

            def qkproj_thunks(p, qcs=range(4), state=None, preload=True):
                """fp8 DoubleRow Q/K projection of pair p over q-chunks qcs."""
                thunks = []
                if state is None:
                    state = {}
                if preload:
                    def pre(p=p):
                        wqk = wqk_sb[:, p * 4096:(p + 1) * 4096]
                        state["q"] = (wqk[:, 0:1024], wqk[:, 1024:2048])
                        state["k"] = (wqk[:, 2048:3072], wqk[:, 3072:4096])
                    thunks.append(pre)

                for key, dest in (("q", qt_sb), ("k", kt_sb)):
                    for qc in qcs:
                        for ki in range(8):
                            def mm(key=key, qc=qc, ki=ki):
                                if ki == 0:
                                    state["acc"] = psp.tile(
                                        [128, 512], f32, tag="projacc",
                                        name="projacc")
                                nc.tensor.matmul(
                                    state["acc"][:],
                                    state[key][0][:, ki * 128:(ki + 1) * 128]
                                    .rearrange("p (o m) -> p o m", o=1)
                                    .broadcast_to([128, 2, 128]),
                                    xhl_s[:, :, ki * 2048 + qc * 512:
                                          ki * 2048 + (qc + 1) * 512],
                                    start=(ki == 0), stop=False, perf_mode=DR)
                            thunks.append(mm)
                        for kd in range(4):
                            def mm(key=key, qc=qc, kd=kd):
                                nc.tensor.matmul(
                                    state["acc"][:],
                                    state[key][1].rearrange(
                                        "p (d s m) -> p d s m",
                                        s=2, m=128)[:, kd],
                                    xhl_a[:, 2 * kd:2 * kd + 2,
                                          qc * 512:(qc + 1) * 512],
                                    start=False, stop=(kd == 3), perf_mode=DR)
                            thunks.append(mm)

                        def cp(dest=dest, qc=qc, p=p):
                            with nc.allow_low_precision(reason="f16 qkv"):
                                nc.vector.tensor_scalar_mul(
                                    dest[:, p * S + qc * 512:
                                         p * S + (qc + 1) * 512],
                                    state["acc"][:], 1.0 / WSCALE)
                        thunks.append(cp)
                return thunks

            def outproj_thunks(qt, js=range(4)):
                """Out-projection of q16 tiles qt*4+j for j in js."""
                thunks = []
                state = {}
                for j in js:
                    q16 = qt * 4 + j

                    def alloc(q16=q16):
                        state["pso"] = [
                            psp.tile([128, 512], f32, tag="projacc",
                                     name=f"pso{fc}") for fc in range(2)]
                        state["osb"] = osbp.tile([128, 1024], f16, tag="osb",
                                                 bufs=4, name="osb")
                    thunks.append(alloc)

                    def at2(src, pp, q16):
                        """[128, 2, 128] DoubleRow slots over pairs pp, pp+1."""
                        return src.rearrange("p (e c) -> p e c", e=4) \
                            [:, pp:pp + 2, q16 * 128:(q16 + 1) * 128]

                    for term, src, wsb in (("hh", ath_sb, woh_sb),
                                           ("lh", atl_sb, woh_sb),
                                           ("hl", ath_sb, wol_sb)):
                        for pp in (0, 2):
                            for fc in range(2):
                                def mm(term=term, src=src, wsb=wsb,
                                       pp=pp, fc=fc, q16=q16):
                                    nc.tensor.matmul(
                                        state["pso"][fc][:],
                                        at2(src, pp, q16),
                                        wsb.rearrange(
                                            "p (e c) -> p e c",
                                            e=4)[:, pp:pp + 2,
                                                 fc * 512:(fc + 1) * 512],
                                        start=(term == "hh" and pp == 0),
                                        stop=(term == "hl" and pp == 2),
                                        perf_mode=DR)
                                thunks.append(mm)

                    def cp(q16=q16):
                        for fc in range(2):
                            with nc.allow_low_precision(reason="descale"):
                                nc.vector.tensor_scalar_mul(
                                    state["osb"][:, fc * 512:(fc + 1) * 512],
                                    state["pso"][fc][:],
                                    1.0 / (ASCALE * WSCALE))
                        nc.sync.dma_start(
                            out_ex[q16 * 128:(q16 + 1) * 128, :], state["osb"][:])
                    thunks.append(cp)
                return thunks

            # ---------- bootstrap: V(0-3) + pair-0 qc0 ----------
            p0_state = {}
            with nc.named_scope("ph2_boot"):
                boot = []
                for pos in range(4):
                    boot += vproj8_thunks(pos)
                for t in qkproj_thunks(0, qcs=[0], state=p0_state)[:1]:
                    t()  # pair-0 weight DMAs first (ahead of x chunks 1-3)
                # seq chunks 1-3 of xh/xl (strided 3D copies), with the
                # remaining weight blocks interleaved by first consumer
                for c in range(1, 4):
                    for s in range(2):
                        nc.sync.dma_start(
                            xhl_s[:, s].rearrange("p (k b) -> p k b", k=8)
                            [:, :, c * 512:(c + 1) * 512],
                            xhl_in.rearrange("p (s k b) -> p s k b",
                                             s=2, b=S)[:, s, :,
                                                       c * 512:(c + 1) * 512])
                    if c == 1:
                        nc.sync.dma_start(wqk_sb[:, 8192:12288],
                                          wqk_in[:, 8192:12288])
                    elif c == 2:
                        nc.sync.dma_start(wqk_sb[:, 12288:16384],
                                          wqk_in[:, 12288:16384])
                    else:
                        nc.sync.dma_start(woh_sb[:], woh_in[:])
                        nc.sync.dma_start(wol_sb[:], wol_in[:])
                for t in boot:
                    t()
                for t in qkproj_thunks(0, qcs=[0], state=p0_state,
                                       preload=False):
                    t()

            # persistent PV accumulators: 4 chains x 65 cols per bank tile.
            # The first matmul of each qt (start=True) zeroes the whole psum
            # bank region, so all four chains start from 0 with no memset.
            pvq = [psp.tile([128, 512], f32, tag=f"pvq{i}", name=f"pvq{i}",
                            bufs=1) for i in range(2)]

            with nc.named_scope("ph3_attn"):
                for p in range(NPAIR):
                    if p + 1 < NPAIR:
                        filler = qkproj_thunks(p + 1)
                    else:
                        filler = []  # pair 3: out-proj of prior qt is filler
                    fi = 0
                    prev_qt = None
                    for qt in range(NQT):
                        if p == 0 and qt > 0:
                            # feed pair-0's own pipeline: next V tiles + qc
                            for t in vproj8_thunks(4 * qt):
                                t()
                            for t in vproj8_thunks(4 * qt + 1):
                                t()
                            for t in vproj8_thunks(4 * qt + 2):
                                t()
                            for t in vproj8_thunks(4 * qt + 3):
                                t()
                            for t in qkproj_thunks(0, qcs=[qt],
                                                   state=p0_state,
                                                   preload=False):
                                t()
                        if p == NPAIR - 1:
                            if prev_qt is not None:
                                filler = filler[fi:] + outproj_thunks(prev_qt)
                            else:
                                filler = []
                            fi = 0
                        prev_qt = qt
                        qs = qt * 512
                        nkt = qs // 128 + 4
                        # pair 3's filler (out-proj) waits on the previous
                        # qt's at_sb transpose: hold it back 2 slots
                        skip_slots = 2 if p == NPAIR - 1 else 0
                        n_slots = nkt + 4 - skip_slots
                        rate = max(0, len(filler) - fi) / n_slots
                        facc = 0.0
                        fired = 0
                        rc = attp.tile([128, 8], f32, tag="rc", bufs=6,
                                       name="rc")
                        an = attp.tile([128, 512], f16, tag="an", bufs=6,
                                       name="an")

                        def softmax_tail(i):
                            """reciprocal + normalize of pvq[i] (qsubs 2i,
                            2i+1) into the an tile, then transpose out its
                            half of at_sb so consumers start early."""
                            nc.vector.reciprocal(
                                rc.rearrange("p (i g o) -> p i g o",
                                             i=2, o=1)[:, i],
                                pvq[i][:, 0:260]
                                .rearrange("p (g c) -> p g c", c=65)[:, :, 64:65])
                            for qs2 in range(2):
                                qsub = 2 * i + qs2
                                base = i * 4 + qs2 * 2
                                with nc.allow_low_precision(reason="f16 attn"):
                                    # both heads in one op: per-(partition,h)
                                    # reciprocal enters as a 0-stride bcast
                                    nc.vector.tensor_mul(
                                        an[:, qsub * 128:(qsub + 1) * 128]
                                        .rearrange("p (s d) -> p s d", s=2),
                                        pvq[i][:, qs2 * 130: qs2 * 130 + 130]
                                        .rearrange("p (s c) -> p s c",
                                                   c=65)[:, :, 0:64],
                                        rc[:, base:base + 2]
                                        .rearrange("p (s o) -> p s o", o=1)
                                        .broadcast_to([128, 2, 64]))
                            cols = slice(p * S + qs + i * 256,
                                         p * S + qs + i * 256 + 256)
                            nc.sync.dma_start(
                                at_sb[:, cols].rearrange("d (e j) -> d e j",
                                                         e=2),
                                an[:, i * 256:(i + 1) * 256], transpose=True)
                            # fp8 hi/lo split for the out-projection (Pool)
                            with nc.allow_low_precision(reason="fp8 at"):
                                nc.gpsimd.tensor_copy(ath_sb[:, cols],
                                                      at_sb[:, cols])
                                nc.gpsimd.tensor_sub(atl_sb[:, cols],
                                                     at_sb[:, cols],
                                                     ath_sb[:, cols])

                        def pv_bank(i, ktm):
                            """PV chains of bank i (qsubs 2i, 2i+1) for ktm."""
                            ptile, c0 = ptiles[ktm]
                            for h in range(2):
                                for qsub in range(max(2 * i, c0 // 128),
                                                  2 * i + 2):
                                    po = (qsub % 2) * 130 + h * 65
                                    nc.tensor.matmul(
                                        pvq[i][:, po:po + 65],
                                        ptile[:, h * 512 + qsub * 128:
                                              h * 512 + (qsub + 1) * 128],
                                        v_sb[:, ktm * 520 + (2 * p + h) * 65:
                                             ktm * 520 + (2 * p + h) * 65 + 65],
                                        start=(ktm == 0 and h == 0
                                               and qsub == 2 * i),
                                        stop=(h == 1 and qsub == 2 * i + 1
                                              and ktm == qs // 128 + qsub))

                        ptiles = {}
                        for kt in range(nkt + 4):
                            if kt < nkt:
                                dlt = kt * 128 - qs
                                c0 = BAND[dlt] if dlt >= 0 else 0
                                sc = psp.tile([128, 1024], f32, tag="sc",
                                              name="sc")
                                for h in range(2):
                                    nc.tensor.matmul(
                                        sc[:, h * 512 + c0: (h + 1) * 512],
                                        kt_sb[h * 64:(h + 1) * 64,
                                              p * S + kt * 128: p * S + (kt + 1) * 128],
                                        qt_sb[h * 64:(h + 1) * 64,
                                              p * S + qs + c0: p * S + qs + 512],
                                        start=True, stop=True)
                                ptile = attp.tile([128, 1024], f16, tag="pt",
                                                  bufs=8, name="pt")
                                sc_v = sc.rearrange("k (h q) -> k h q", h=2)[:, :, c0:512]
                                pt_v = ptile.rearrange("k (h q) -> k h q", h=2)[:, :, c0:512]
                                with nc.allow_low_precision(reason="f16 probs"):
                                    nc.scalar.activation(
                                        pt_v, sc_v, mybir.ActivationFunctionType.Exp,
                                        bias=0.0, scale=float(SCALE))
                                if dlt >= 0:
                                    blk = ptile.rearrange(
                                        "k (h q) -> k h q",
                                        h=2)[:, :, dlt:dlt + 128]
                                    nc.gpsimd.affine_select(
                                        out=blk, in_=blk,
                                        compare_op=mybir.AluOpType.is_ge,
                                        fill=0.0, base=0,
                                        pattern=[[0, 2], [1, 128]],
                                        channel_multiplier=-1)
                                ptiles[kt] = (ptile, c0)
                            if 2 <= kt < nkt + 2:   # bank 0: PV for kt-2
                                ktm = kt - 2
                                if ptiles[ktm][1] // 128 < 2:
                                    pv_bank(0, ktm)
                                if ktm == nkt - 3:
                                    softmax_tail(0)  # qsubs 0,1 complete
                            if kt >= 4:             # bank 1: PV for kt-4
                                ktm = kt - 4
                                pv_bank(1, ktm)
                                ptiles.pop(ktm)
                            if kt >= skip_slots:
                                facc += rate
                                while fired < int(facc) and fi < len(filler):
                                    filler[fi]()
                                    fi += 1
                                    fired += 1
                        # tail: qsubs 2,3 normalize + transpose
                        softmax_tail(1)
                    # flush remaining filler at end of pair
                    for t in filler[fi:]:
                        t()
                    fi = len(filler)

            # ---------- phase 4: final out projection chunk ----------
            with nc.named_scope("ph4_outproj"):
                for t in outproj_thunks(NQT - 1):
                    t()

    nc.compile()
    return nc


def _get_program():
    if "nc" not in _CACHE:
        _CACHE["nc"] = _build()
    return _CACHE["nc"]


def _split8(w):
    import ml_dtypes
    f8 = ml_dtypes.float8_e4m3
    h = w.astype(f8)
    l = (w - h.astype(np.float32)).astype(f8)
    return h, l


def _make_in_maps(x, w_qkv, w_out):
    import ml_dtypes
    f8np = ml_dtypes.float8_e4m3
    x = np.asarray(x, dtype=np.float32).astype(np.float16)
    w_qkv = np.asarray(w_qkv, dtype=np.float32)
    w_out = np.asarray(w_out, dtype=np.float32)

    w_q, w_k, w_v = w_qkv[:, 0:DM], w_qkv[:, DM:2 * DM], w_qkv[:, 2 * DM:3 * DM]

    def pack_x(xb16):
        """[2048, 1024] fp16 -> [128, 2*8*2048] fp8 hi/lo, transposed."""
        xh = xb16.astype(f8np)
        xl = (xb16.astype(np.float32) - xh.astype(np.float32)).astype(f8np)

        def tr(part):
            return part.T.reshape(8, 128, S).transpose(1, 0, 2).reshape(128, 8 * S)
        return np.ascontiguousarray(
            np.concatenate([tr(xh), tr(xl)], axis=1))

    xhl_packs = [pack_x(x[b]) for b in range(B)]

    def pack_qk(w_core):
        """[1024, 512] -> hi [128, 4*2048], lo [128, 4*1024] DoubleRow packs."""
        w_core = w_core * WSCALE
        his, los = [], []
        for p in range(NPAIR):
            h, l = _split8(w_core[:, p * 128:(p + 1) * 128])
            # hi: [r, ki, m] (DoubleRow dup comes from a 0-stride broadcast)
            hp = h.reshape(8, 128, 128).transpose(1, 0, 2)
            his.append(np.ascontiguousarray(hp.reshape(128, 1024)))
            # lo: [r, kd, s, m] with s the ki sub-chunk
            lp = l.reshape(4, 2, 128, 128).transpose(2, 0, 1, 3)
            los.append(lp.reshape(128, 1024))
        return np.concatenate(his, 1), np.concatenate(los, 1)

    def pack_v(w_core):
        """[1024, 512] -> hi [128, 8*512], lo [128, 4*1024]."""
        h, l = _split8(w_core * WSCALE)
        hp = h.reshape(8, 128, 512).transpose(1, 0, 2)
        lp = l.reshape(4, 2, 128, 512).transpose(2, 0, 1, 3)
        return (np.ascontiguousarray(hp.reshape(128, 4096)),
                np.ascontiguousarray(lp.reshape(128, 4096)))

    in_maps = []
    for c in range(8):
        b, s = c // 2, c % 2
        cols = slice(s * HPC * HS, (s + 1) * HPC * HS)
        wqh, wql = pack_qk(w_q[:, cols])
        wkh, wkl = pack_qk(w_k[:, cols])
        # one [qh|ql|kh|kl] block per pair
        wqk = np.concatenate(
            [np.concatenate([wqh[:, p * 1024:(p + 1) * 1024],
                             wql[:, p * 1024:(p + 1) * 1024],
                             wkh[:, p * 1024:(p + 1) * 1024],
                             wkl[:, p * 1024:(p + 1) * 1024]], axis=1)
             for p in range(NPAIR)], axis=1)
        wvh, wvl = pack_v(w_v[:, cols])
        # wo: [512, 1024] scaled, fp8 hi/lo, [r, pair, c] layout
        woh8, wol8 = _split8(w_out[cols, :] * WSCALE)
        woh = np.ascontiguousarray(
            woh8.reshape(4, 128, 1024).transpose(1, 0, 2).reshape(128, 4096))
        wol = np.ascontiguousarray(
            wol8.reshape(4, 128, 1024).transpose(1, 0, 2).reshape(128, 4096))
        in_maps.append({
            "xhl_in": xhl_packs[b],
            "woh_in": woh, "wol_in": wol,
            "wqk_in": np.ascontiguousarray(wqk),
            "wvh_in": wvh, "wvl_in": wvl,
        })
    return in_maps


def kernel(x: np.ndarray, w_qkv: np.ndarray, w_out: np.ndarray) -> np.ndarray:
    from concourse.bass_utils import run_bass_kernel_spmd

    nc = _get_program()
    in_maps = _make_in_maps(x, w_qkv, w_out)
    res = run_bass_kernel_spmd(nc, in_maps, core_ids=list(range(8)))
    out = np.empty((B, S, DM), dtype=np.float32)
    for b in range(B):
        out[b] = (res.results[2 * b]["out_partial"].astype(np.float32)
                  + res.results[2 * b + 1]["out_partial"].astype(np.float32))
    return out



# revision 13
# speedup vs baseline: 1.1541x; 1.1069x over previous
"""Multi-head causal attention (B=4, S=2048, D=1024, H=16) on 8 Trainium2 cores.

Sharding: core c handles batch c//2 and heads [8*(c%2), 8*(c%2)+8).
Each core computes a partial output (its 8 heads' contribution to all 2048
rows of its batch); the host sums the two partials per batch. No collectives.

Pipeline (fp16 operands, fp8e4m3 hi/lo DoubleRow matmuls, f32 PSUM):
  x    arrives pre-transposed and hi/lo fp8-split from the host (xhl_in);
         weights arrive fp8 hi/lo, pre-scaled by 64 so the lo residuals
         stay out of e4m3 subnormals, packed in DoubleRow slot layout
  Q/K/V projections: DoubleRow fp8 matmuls at 0.5 cycles/row,
         w^T x ~= wh^T(xh+xl) + wl^T xh (relative error ~0.1%); the
         un-duplicated hi weights enter via 0-stride broadcast APs
  scores^T = Kt^T Qt per (pair, 512-q tile, 128-k tile), band-narrowed
  P^T  = exp(scores/8) on ACT, causal diag masked in place by one strided
         gpsimd affine_select per k-tile (both heads at once)
  PV flipped: attn[q,65] += P^T[k,qsub]^T V[k,65] (psum bank = 4 chains
         behind one start=True region-zero; the two banks are software-
         pipelined 2 kt-iterations apart so softmax tails overlap PV)
  column 64 of V holds 1/32 -> per-partition softmax sums -> DVE
         reciprocal + tensor_scalar normalize (at_sb holds 32*attn);
         attn_n -> at_sb via one DMA-crossbar transpose per half-qt,
         then Pool casts the fp8 hi/lo copy used by the out-projection
  out-proj (at^T w_out) in fp8 DoubleRow, interleaved into pair-3's
         attention as filler; partials summed on the host per batch

Q/K projections of pair p+1 are interleaved as filler into the attention
of pair p (fractionally paced); the out projection is the filler of
pair 3. Startup DMAs are few and large (HWDGE issue overhead dominates
small transfers) and ordered by first consumer.
"""
import sys
sys.path.insert(0, '/opt/trn_rl_repo')

import numpy as np

B, S, DM, H, HS = 4, 2048, 1024, 16, 64
HPC = 8      # heads per core
NPAIR = 4    # head pairs per core
NQT = 4      # q tiles of 512
SCALE = 1.0 / np.sqrt(HS)
WSCALE = 64.0  # fp8 hi/lo weight pre-scale (keeps residuals out of subnormals)
ASCALE = 32.0  # fp8 hi/lo pre-scale for the attention output path

_CACHE = {}


def _build():
    import concourse.bass as bass
    import concourse.bacc as bacc
    import concourse.mybir as mybir
    from concourse.tile import TileContext

    f32 = mybir.dt.float32
    f16 = mybir.dt.float16
    f8 = mybir.dt.float8e4
    DR = mybir.MatmulPerfMode.DoubleRow

    nc = bacc.Bacc()
    # x pre-transposed and hi/lo fp8-split host-side:
    # col = s*16384 + ki*2048 + seq, row = dm%128 (ki = dm//128)
    xhl_in = nc.dram_tensor("xhl_in", [128, 2 * 8 * S], f8, kind="ExternalInput")
    woh_in = nc.dram_tensor("woh_in", [128, 4 * 1024], f8, kind="ExternalInput")
    wol_in = nc.dram_tensor("wol_in", [128, 4 * 1024], f8, kind="ExternalInput")
    # fp8 hi/lo Q/K weights, one [qh|ql|kh|kl] block per pair so each
    # pair preload is a single DMA
    wqk_in = nc.dram_tensor("wqk_in", [128, 4 * 4096], f8, kind="ExternalInput")
    wvh_in = nc.dram_tensor("wvh_in", [128, 8 * 512], f8, kind="ExternalInput")
    wvl_in = nc.dram_tensor("wvl_in", [128, 4 * 1024], f8, kind="ExternalInput")
    out_ex = nc.dram_tensor("out_partial", [S, DM], f16, kind="ExternalOutput")

    # band plan: for band k-tile with offset dlt, compute cols [c0, 512)
    BAND = {0: 0, 128: 128, 256: 256, 384: 384}

    with TileContext(nc) as tc:
        with tc.tile_pool(name="persist", bufs=1) as persist, \
             tc.tile_pool(name="prj", bufs=4) as prj, \
             tc.tile_pool(name="attp", bufs=2) as attp, \
             tc.tile_pool(name="osbp", bufs=2) as osbp, \
             tc.tile_pool(name="psp", bufs=2, space="PSUM") as psp:

            qt_sb = persist.tile([128, NPAIR * S], f16)          # 2 MB
            kt_sb = persist.tile([128, NPAIR * S], f16)          # 2 MB
            v_sb = persist.tile([128, 16 * HPC * 65], f16)       # 2.1 MB
            at_sb = persist.tile([128, NPAIR * S], f16)          # 2 MB
            ath_sb = persist.tile([128, NPAIR * S], f8)          # 1 MB
            atl_sb = persist.tile([128, NPAIR * S], f8)          # 1 MB
            woh_sb = persist.tile([128, 4 * 1024], f8)           # 0.5 MB
            wol_sb = persist.tile([128, 4 * 1024], f8)           # 0.5 MB
            wvh_sb = persist.tile([128, 8 * 512], f8)            # 0.5 MB
            wvl_sb = persist.tile([128, 4 * 1024], f8)           # 0.5 MB
            wqk_sb = persist.tile([128, 4 * 4096], f8)           # 2 MB
            # xh/xl fp8: col = s*16384 + ki*2048 + seq
            xhl = persist.tile([128, 2 * 8 * S], f8)             # 4 MB
            xhl_s = xhl.rearrange("p (s r) -> p s r", s=2)
            xhl_a = xhl.rearrange("p (a b) -> p a b", b=S)

            # ---------- startup DMAs, ordered by first consumer ----------
            with nc.named_scope("ph1_load"):
                # few large transfers: HWDGE issue overhead dominates many
                # small ones; order tracks the V-proj ki consumption
                def c0_piece(s, k0, k1):
                    view = (lambda t: t.rearrange("p (s2 k b) -> p s2 k b",
                                                  s2=2, b=S))
                    nc.sync.dma_start(
                        view(xhl)[:, s, k0:k1, 0:512],
                        view(xhl_in)[:, s, k0:k1, 0:512])
                # small lead slices let V-proj pos0 start ~3us sooner;
                # the remainder stays in large transfers
                nc.sync.dma_start(wvh_sb[:, 0:1024], wvh_in[:, 0:1024])
                c0_piece(0, 0, 2)
                c0_piece(1, 0, 2)
                nc.sync.dma_start(wvh_sb[:, 1024:4096], wvh_in[:, 1024:4096])
                c0_piece(0, 2, 4)
                c0_piece(1, 2, 4)
                nc.sync.dma_start(wvl_sb[:], wvl_in[:])
                c0_piece(0, 4, 8)
                c0_piece(1, 4, 8)
                nc.sync.dma_start(wqk_sb[:, 0:4096], wqk_in[:, 0:4096])
                nc.sync.dma_start(wqk_sb[:, 4096:8192], wqk_in[:, 4096:8192])

            # "ones" columns of v_sb at 1/WSCALE: sums come out pre-divided,
            # so the reciprocal yields WSCALE/sum and at_sb holds WSCALE*attn
            # (keeps the fp8 hi/lo split of at_sb out of subnormals)
            for pos in range(16):
                nc.vector.memset(
                    v_sb[:, pos * 520:(pos + 1) * 520]
                    .rearrange("p (h c) -> p h c", c=65)[:, :, 64:65],
                    1.0 / ASCALE)

            # ---------- projection emitters ----------
            def vproj8_thunks(pos):
                """fp8 DoubleRow V projection of seq tile pos as thunks."""
                thunks = []
                state = {}

                def alloc():
                    state["psv"] = psp.tile([128, 512], f32, tag="projacc",
                                            name="psv")
                thunks.append(alloc)
                for ki in range(8):
                    def mm(ki=ki):
                        nc.tensor.matmul(
                            state["psv"][:],
                            xhl_s[:, :, ki * 2048 + pos * 128:
                                  ki * 2048 + pos * 128 + 128],
                            wvh_sb[:, ki * 512:(ki + 1) * 512]
                            .rearrange("p (o c) -> p o c", o=1)
                            .broadcast_to([128, 2, 512]),
                            start=(ki == 0), stop=False, perf_mode=DR)
                    thunks.append(mm)
                for kd in range(4):
                    def mm(kd=kd):
                        nc.tensor.matmul(
                            state["psv"][:],
                            xhl_a[:, 2 * kd:2 * kd + 2,
                                  pos * 128:pos * 128 + 128],
                            wvl_sb.rearrange("p (d s c) -> p d s c",
                                             s=2, c=512)[:, kd],
                            start=False, stop=(kd == 3), perf_mode=DR)
                    thunks.append(mm)

                def cp():
                    with nc.allow_low_precision(reason="f16 V"):
                        nc.vector.tensor_scalar_mul(
                            v_sb[:, pos * 520: (pos + 1) * 520]
                            .rearrange("p (h c) -> p h c", c=65)[:, :, 0:64],
                            state["psv"].rearrange("p (h c) -> p h c", c=64),
                            1.0 / WSCALE)
                thunks.append(cp)
                return thunks

            def qkproj_thunks(p, qcs=range(4), state=None, preload=True):
                """fp8 DoubleRow Q/K projection of pair p over q-chunks qcs."""
                thunks = []
                if state is None:
                    state = {}
                if preload:
                    def pre(p=p):
                        wqk = wqk_sb[:, p * 4096:(p + 1) * 4096]
                        state["q"] = (wqk[:, 0:1024], wqk[:, 1024:2048])
                        state["k"] = (wqk[:, 2048:3072], wqk[:, 3072:4096])
                    thunks.append(pre)

                for key, dest in (("q", qt_sb), ("k", kt_sb)):
                    for qc in qcs:
                        for ki in range(8):
                            def mm(key=key, qc=qc, ki=ki):
                                if ki == 0:
                                    state["acc"] = psp.tile(
                                        [128, 512], f32, tag="projacc",
                                        name="projacc")
                                nc.tensor.matmul(
                                    state["acc"][:],
                                    state[key][0][:, ki * 128:(ki + 1) * 128]
                                    .rearrange("p (o m) -> p o m", o=1)
                                    .broadcast_to([128, 2, 128]),
                                    xhl_s[:, :, ki * 2048 + qc * 512:
                                          ki * 2048 + (qc + 1) * 512],
                                    start=(ki == 0), stop=False, perf_mode=DR)
                            thunks.append(mm)
                        for kd in range(4):
                            def mm(key=key, qc=qc, kd=kd):
                                nc.tensor.matmul(
                                    state["acc"][:],
                                    state[key][1].rearrange(
                                        "p (d s m) -> p d s m",
                                        s=2, m=128)[:, kd],
                                    xhl_a[:, 2 * kd:2 * kd + 2,
                                          qc * 512:(qc + 1) * 512],
                                    start=False, stop=(kd == 3), perf_mode=DR)
                            thunks.append(mm)

                        def cp(dest=dest, qc=qc, p=p):
                            with nc.allow_low_precision(reason="f16 qkv"):
                                nc.vector.tensor_scalar_mul(
                                    dest[:, p * S + qc * 512:
                                         p * S + (qc + 1) * 512],
                                    state["acc"][:], 1.0 / WSCALE)
                        thunks.append(cp)
                return thunks

            def outproj_thunks(qt, js=range(4)):
                """Out-projection of q16 tiles qt*4+j for j in js."""
                thunks = []
                state = {}
                for j in js:
                    q16 = qt * 4 + j

                    def alloc(q16=q16):
                        state["pso"] = [
                            psp.tile([128, 512], f32, tag="projacc",
                                     name=f"pso{fc}") for fc in range(2)]
                        state["osb"] = osbp.tile([128, 1024], f16, tag="osb",
                                                 bufs=4, name="osb")
                    thunks.append(alloc)

                    def at2(src, pp, q16):
                        """[128, 2, 128] DoubleRow slots over pairs pp, pp+1."""
                        return src.rearrange("p (e c) -> p e c", e=4) \
                            [:, pp:pp + 2, q16 * 128:(q16 + 1) * 128]

                    for term, src, wsb in (("hh", ath_sb, woh_sb),
                                           ("lh", atl_sb, woh_sb),
                                           ("hl", ath_sb, wol_sb)):
                        for pp in (0, 2):
                            for fc in range(2):
                                def mm(term=term, src=src, wsb=wsb,
                                       pp=pp, fc=fc, q16=q16):
                                    nc.tensor.matmul(
                                        state["pso"][fc][:],
                                        at2(src, pp, q16),
                                        wsb.rearrange(
                                            "p (e c) -> p e c",
                                            e=4)[:, pp:pp + 2,
                                                 fc * 512:(fc + 1) * 512],
                                        start=(term == "hh" and pp == 0),
                                        stop=(term == "hl" and pp == 2),
                                        perf_mode=DR)
                                thunks.append(mm)

                    def cp(q16=q16):
                        for fc in range(2):
                            with nc.allow_low_precision(reason="descale"):
                                nc.vector.tensor_scalar_mul(
                                    state["osb"][:, fc * 512:(fc + 1) * 512],
                                    state["pso"][fc][:],
                                    1.0 / (ASCALE * WSCALE))
                        nc.sync.dma_start(
                            out_ex[q16 * 128:(q16 + 1) * 128, :], state["osb"][:])
                    thunks.append(cp)
                return thunks

            # per-pair stationary-weight slices out of the persistent wqk_sb
            pstates = {}
            for pp_ in range(NPAIR):
                wp_ = wqk_sb[:, pp_ * 4096:(pp_ + 1) * 4096]
                pstates[pp_] = {"q": (wp_[:, 0:1024], wp_[:, 1024:2048]),
                                "k": (wp_[:, 2048:3072], wp_[:, 3072:4096])}

            # ---------- bootstrap: V(0-3) + pair-0 qc0 ----------
            with nc.named_scope("ph2_boot"):
                boot = []
                for pos in range(4):
                    boot += vproj8_thunks(pos)
                # seq chunks 1-3 of xh/xl (strided 3D copies), with the
                # remaining weight blocks interleaved by first consumer
                for c in range(1, 4):
                    for s in range(2):
                        nc.sync.dma_start(
                            xhl_s[:, s].rearrange("p (k b) -> p k b", k=8)
                            [:, :, c * 512:(c + 1) * 512],
                            xhl_in.rearrange("p (s k b) -> p s k b",
                                             s=2, b=S)[:, s, :,
                                                       c * 512:(c + 1) * 512])
                    if c == 1:
                        nc.sync.dma_start(wqk_sb[:, 8192:12288],
                                          wqk_in[:, 8192:12288])
                    elif c == 2:
                        nc.sync.dma_start(wqk_sb[:, 12288:16384],
                                          wqk_in[:, 12288:16384])
                    else:
                        nc.sync.dma_start(woh_sb[:], woh_in[:])
                        nc.sync.dma_start(wol_sb[:], wol_in[:])
                for t in boot:
                    t()
                for t in qkproj_thunks(0, qcs=[0], state=pstates[0],
                                       preload=False):
                    t()

            # persistent PV accumulators: 4 chains x 65 cols per bank tile.
            # The first matmul of each qt (start=True) zeroes the whole psum
            # bank region, so all four chains start from 0 with no memset.
            pvq = [psp.tile([128, 512], f32, tag=f"pvq{i}", name=f"pvq{i}",
                            bufs=1) for i in range(2)]

            # ---------- global filler stream (deadline-paced) ----------
            # blocks run qt-major: b = 4*qt + p.  All projection work forms
            # one FIFO of units; unit deadlines say which block needs them.
            # Units drain into the ACT-surplus of earlier slots (paced) and
            # are force-drained at their deadline block.
            fillerq = []  # [deadline_block, thunks, cursor]

            def push_unit(deadline, thunks):
                if thunks:
                    fillerq.append([deadline, list(thunks), 0])

            def filler_fire(n):
                fired = 0
                while fired < n and fillerq:
                    u = fillerq[0]
                    u[1][u[2]]()
                    u[2] += 1
                    fired += 1
                    if u[2] >= len(u[1]):
                        fillerq.pop(0)
                return fired

            def filler_due(maxdl):
                return sum(len(u[1]) - u[2] for u in fillerq if u[0] <= maxdl)

            def drain_due(b):
                while fillerq and fillerq[0][0] <= b:
                    u = fillerq[0]
                    while u[2] < len(u[1]):
                        u[1][u[2]]()
                        u[2] += 1
                    fillerq.pop(0)

            for rnd in range(NQT):
                for fp in range(1, NPAIR):
                    push_unit(4 * rnd + fp,
                              qkproj_thunks(fp, qcs=[rnd], state=pstates[fp],
                                            preload=False))
                if rnd + 1 < NQT:
                    push_unit(4 * (rnd + 1),
                              qkproj_thunks(0, qcs=[rnd + 1],
                                            state=pstates[0], preload=False))
                    for j in range(4):
                        push_unit(4 * (rnd + 1),
                                  vproj8_thunks(4 * (rnd + 1) + j))

            with nc.named_scope("ph3_attn"):
                from collections import deque
                carry = deque()   # per-slot post-lists left over from the
                                  # previous block (trailing PV + tails)
                for qt in range(NQT):
                  for p in range(NPAIR):
                    b = 4 * qt + p
                    drain_due(b)
                    qs = qt * 512
                    nkt = qs // 128 + 4
                    rc = attp.tile([128, 8], f32, tag="rc", bufs=6,
                                   name="rc")
                    an = attp.tile([128, 512], f16, tag="an", bufs=6,
                                   name="an")
                    ptiles = {}

                    def softmax_tail(i, p=p, qs=qs, rc=rc, an=an):
                        """reciprocal + normalize of pvq[i] (qsubs 2i,
                        2i+1) into the an tile, then transpose out its
                        half of at_sb so consumers start early."""
                        nc.vector.reciprocal(
                            rc.rearrange("p (i g o) -> p i g o",
                                         i=2, o=1)[:, i],
                            pvq[i][:, 0:260]
                            .rearrange("p (g c) -> p g c", c=65)[:, :, 64:65])
                        for qs2 in range(2):
                            qsub = 2 * i + qs2
                            base = i * 4 + qs2 * 2
                            with nc.allow_low_precision(reason="f16 attn"):
                                # both heads in one op: per-(partition,h)
                                # reciprocal enters as a 0-stride bcast
                                nc.vector.tensor_mul(
                                    an[:, qsub * 128:(qsub + 1) * 128]
                                    .rearrange("p (s d) -> p s d", s=2),
                                    pvq[i][:, qs2 * 130: qs2 * 130 + 130]
                                    .rearrange("p (s c) -> p s c",
                                               c=65)[:, :, 0:64],
                                    rc[:, base:base + 2]
                                    .rearrange("p (s o) -> p s o", o=1)
                                    .broadcast_to([128, 2, 64]))
                        cols = slice(p * S + qs + i * 256,
                                     p * S + qs + i * 256 + 256)
                        nc.sync.dma_start(
                            at_sb[:, cols].rearrange("d (e j) -> d e j",
                                                     e=2),
                            an[:, i * 256:(i + 1) * 256], transpose=True)
                        # fp8 hi/lo split for the out-projection (Pool)
                        with nc.allow_low_precision(reason="fp8 at"):
                            nc.gpsimd.tensor_copy(ath_sb[:, cols],
                                                  at_sb[:, cols])
                            nc.gpsimd.tensor_sub(atl_sb[:, cols],
                                                 at_sb[:, cols],
                                                 ath_sb[:, cols])

                    def pv_bank(i, ktm, p=p, qs=qs, ptiles=ptiles):
                        """PV chains of bank i (qsubs 2i, 2i+1) for ktm."""
                        ptile, c0 = ptiles[ktm]
                        for h in range(2):
                            for qsub in range(max(2 * i, c0 // 128),
                                              2 * i + 2):
                                po = (qsub % 2) * 130 + h * 65
                                nc.tensor.matmul(
                                    pvq[i][:, po:po + 65],
                                    ptile[:, h * 512 + qsub * 128:
                                          h * 512 + (qsub + 1) * 128],
                                    v_sb[:, ktm * 520 + (2 * p + h) * 65:
                                         ktm * 520 + (2 * p + h) * 65 + 65],
                                    start=(ktm == 0 and h == 0
                                           and qsub == 2 * i),
                                    stop=(h == 1 and qsub == 2 * i + 1
                                          and ktm == qs // 128 + qsub))

                    def posts_for(kt, nkt=nkt, qt=qt, p=p, ptiles=ptiles,
                                  pv_bank=pv_bank, softmax_tail=softmax_tail):
                        ts = []
                        if 2 <= kt < nkt + 2:   # bank 0: PV for kt-2
                            ktm0 = kt - 2
                            def t0(ktm0=ktm0):
                                if ptiles[ktm0][1] // 128 < 2:
                                    pv_bank(0, ktm0)
                                if ktm0 == nkt - 3:
                                    softmax_tail(0)
                            ts.append(t0)
                        if kt >= 4:             # bank 1: PV for kt-4
                            ktm1 = kt - 4
                            def t1(ktm1=ktm1):
                                pv_bank(1, ktm1)
                                ptiles.pop(ktm1)
                            ts.append(t1)
                        if kt == nkt + 3:
                            def t2():
                                softmax_tail(1)
                                if p == NPAIR - 1:
                                    # all 4 pairs of qt are now split to
                                    # fp8: their out-proj becomes filler
                                    push_unit(98, outproj_thunks(qt))
                            ts.append(t2)
                        return ts

                    facc = 0.0
                    ffired = 0
                    for kt in range(nkt):
                        dlt = kt * 128 - qs
                        c0 = BAND[dlt] if dlt >= 0 else 0
                        sc = psp.tile([128, 1024], f32, tag="sc",
                                      name="sc")
                        for h in range(2):
                            nc.tensor.matmul(
                                sc[:, h * 512 + c0: (h + 1) * 512],
                                kt_sb[h * 64:(h + 1) * 64,
                                      p * S + kt * 128: p * S + (kt + 1) * 128],
                                qt_sb[h * 64:(h + 1) * 64,
                                      p * S + qs + c0: p * S + qs + 512],
                                start=True, stop=True)
                        ptile = attp.tile([128, 1024], f16, tag="pt",
                                          bufs=8, name="pt")
                        sc_v = sc.rearrange("k (h q) -> k h q", h=2)[:, :, c0:512]
                        pt_v = ptile.rearrange("k (h q) -> k h q", h=2)[:, :, c0:512]
                        with nc.allow_low_precision(reason="f16 probs"):
                            nc.scalar.activation(
                                pt_v, sc_v, mybir.ActivationFunctionType.Exp,
                                bias=0.0, scale=float(SCALE))
                        if dlt >= 0:
                            blk = ptile.rearrange(
                                "k (h q) -> k h q",
                                h=2)[:, :, dlt:dlt + 128]
                            nc.gpsimd.affine_select(
                                out=blk, in_=blk,
                                compare_op=mybir.AluOpType.is_ge,
                                fill=0.0, base=0,
                                pattern=[[0, 2], [1, 128]],
                                channel_multiplier=-1)
                        ptiles[kt] = (ptile, c0)
                        # previous block's trailing PV overlaps our scores
                        if carry:
                            for t in carry.popleft():
                                t()
                        for t in posts_for(kt):
                            t()
                        # filler pump: fill the ACT surplus of this slot,
                        # and at least keep up with upcoming deadlines
                        act_ns = 1.667 * (512 - c0) + 185
                        pe_ns = 0.833 * (512 - c0) + 250
                        rate = max(act_ns - pe_ns, 0.0) / 107.0
                        rate = max(rate, filler_due(b + 1) / max(nkt - kt, 1))
                        facc += min(rate, 6.0)
                        ffired += filler_fire(int(facc) - ffired)
                    for kt in range(nkt, nkt + 4):
                        carry.append(posts_for(kt))

            # ---------- phase 4: drain trailing PV + final out-proj ----------
            with nc.named_scope("ph4_outproj"):
                while carry:
                    for t in carry.popleft():
                        t()
                drain_due(999)

    nc.compile()
    return nc


def _get_program():
    if "nc" not in _CACHE:
        _CACHE["nc"] = _build()
    return _CACHE["nc"]


def _split8(w):
    import ml_dtypes
    f8 = ml_dtypes.float8_e4m3
    h = w.astype(f8)
    l = (w - h.astype(np.float32)).astype(f8)
    return h, l


def _make_in_maps(x, w_qkv, w_out):
    import ml_dtypes
    f8np = ml_dtypes.float8_e4m3
    x = np.asarray(x, dtype=np.float32).astype(np.float16)
    w_qkv = np.asarray(w_qkv, dtype=np.float32)
    w_out = np.asarray(w_out, dtype=np.float32)

    w_q, w_k, w_v = w_qkv[:, 0:DM], w_qkv[:, DM:2 * DM], w_qkv[:, 2 * DM:3 * DM]

    def pack_x(xb16):
        """[2048, 1024] fp16 -> [128, 2*8*2048] fp8 hi/lo, transposed."""
        xh = xb16.astype(f8np)
        xl = (xb16.astype(np.float32) - xh.astype(np.float32)).astype(f8np)

        def tr(part):
            return part.T.reshape(8, 128, S).transpose(1, 0, 2).reshape(128, 8 * S)
        return np.ascontiguousarray(
            np.concatenate([tr(xh), tr(xl)], axis=1))

    xhl_packs = [pack_x(x[b]) for b in range(B)]

    def pack_qk(w_core):
        """[1024, 512] -> hi [128, 4*2048], lo [128, 4*1024] DoubleRow packs."""
        w_core = w_core * WSCALE
        his, los = [], []
        for p in range(NPAIR):
            h, l = _split8(w_core[:, p * 128:(p + 1) * 128])
            # hi: [r, ki, m] (DoubleRow dup comes from a 0-stride broadcast)
            hp = h.reshape(8, 128, 128).transpose(1, 0, 2)
            his.append(np.ascontiguousarray(hp.reshape(128, 1024)))
            # lo: [r, kd, s, m] with s the ki sub-chunk
            lp = l.reshape(4, 2, 128, 128).transpose(2, 0, 1, 3)
            los.append(lp.reshape(128, 1024))
        return np.concatenate(his, 1), np.concatenate(los, 1)

    def pack_v(w_core):
        """[1024, 512] -> hi [128, 8*512], lo [128, 4*1024]."""
        h, l = _split8(w_core * WSCALE)
        hp = h.reshape(8, 128, 512).transpose(1, 0, 2)
        lp = l.reshape(4, 2, 128, 512).transpose(2, 0, 1, 3)
        return (np.ascontiguousarray(hp.reshape(128, 4096)),
                np.ascontiguousarray(lp.reshape(128, 4096)))

    in_maps = []
    for c in range(8):
        b, s = c // 2, c % 2
        cols = slice(s * HPC * HS, (s + 1) * HPC * HS)
        wqh, wql = pack_qk(w_q[:, cols])
        wkh, wkl = pack_qk(w_k[:, cols])
        # one [qh|ql|kh|kl] block per pair
        wqk = np.concatenate(
            [np.concatenate([wqh[:, p * 1024:(p + 1) * 1024],
                             wql[:, p * 1024:(p + 1) * 1024],
                             wkh[:, p * 1024:(p + 1) * 1024],
                             wkl[:, p * 1024:(p + 1) * 1024]], axis=1)
             for p in range(NPAIR)], axis=1)
        wvh, wvl = pack_v(w_v[:, cols])
        # wo: [512, 1024] scaled, fp8 hi/lo, [r, pair, c] layout
        woh8, wol8 = _split8(w_out[cols, :] * WSCALE)
        woh = np.ascontiguousarray(
            woh8.reshape(4, 128, 1024).transpose(1, 0, 2).reshape(128, 4096))
        wol = np.ascontiguousarray(
            wol8.reshape(4, 128, 1024).transpose(1, 0, 2).reshape(128, 4096))
        in_maps.append({
            "xhl_in": xhl_packs[b],
            "woh_in": woh, "wol_in": wol,
            "wqk_in": np.ascontiguousarray(wqk),
            "wvh_in": wvh, "wvl_in": wvl,
        })
    return in_maps


def kernel(x: np.ndarray, w_qkv: np.ndarray, w_out: np.ndarray) -> np.ndarray:
    from concourse.bass_utils import run_bass_kernel_spmd

    nc = _get_program()
    in_maps = _make_in_maps(x, w_qkv, w_out)
    res = run_bass_kernel_spmd(nc, in_maps, core_ids=list(range(8)))
    out = np.empty((B, S, DM), dtype=np.float32)
    for b in range(B):
        out[b] = (res.results[2 * b]["out_partial"].astype(np.float32)
                  + res.results[2 * b + 1]["out_partial"].astype(np.float32))
    return out



# revision 19
# speedup vs baseline: 1.1955x; 1.0359x over previous
"""Multi-head causal attention (B=4, S=2048, D=1024, H=16) on 8 Trainium2 cores.

Sharding: core c handles batch c//2 and heads [8*(c%2), 8*(c%2)+8).
Each core computes a partial output (its 8 heads' contribution to all 2048
rows of its batch); the host sums the two partials per batch. No collectives.

Pipeline (fp16 operands, fp8e4m3 hi/lo DoubleRow matmuls, f32 PSUM):
  x    arrives pre-transposed and hi/lo fp8-split from the host (xhl_in);
         weights arrive fp8 hi/lo, pre-scaled by 64 so the lo residuals
         stay out of e4m3 subnormals, packed in DoubleRow slot layout
  Q/K/V projections: DoubleRow fp8 matmuls at 0.5 cycles/row,
         w^T x ~= wh^T(xh+xl) + wl^T xh (relative error ~0.1%); the
         un-duplicated hi weights enter via 0-stride broadcast APs
  scores^T = Kt^T Qt per (pair, 512-q tile, 128-k tile), band-narrowed
  P^T  = exp(scores/8) on ACT, causal diag masked in place by one strided
         gpsimd affine_select per k-tile (both heads at once)
  PV flipped: attn[q,65] += P^T[k,qsub]^T V[k,65] (psum bank = 4 chains
         behind one start=True region-zero; the two banks are software-
         pipelined 2 kt-iterations apart so softmax tails overlap PV)
  column 64 of V holds 1/32 -> per-partition softmax sums -> DVE
         reciprocal + tensor_scalar normalize (at_sb holds 32*attn);
         attn_n -> at_sb via one DMA-crossbar transpose per half-qt,
         then Pool casts the fp8 hi/lo copy used by the out-projection
  out-proj (at^T w_out) in fp8 DoubleRow, interleaved into pair-3's
         attention as filler; partials summed on the host per batch

Q/K projections of pair p+1 are interleaved as filler into the attention
of pair p (fractionally paced); the out projection is the filler of
pair 3. Startup DMAs are few and large (HWDGE issue overhead dominates
small transfers) and ordered by first consumer.
"""
import sys
sys.path.insert(0, '/opt/trn_rl_repo')

import numpy as np

B, S, DM, H, HS = 4, 2048, 1024, 16, 64
HPC = 8      # heads per core
NPAIR = 4    # head pairs per core
NQT = 4      # q tiles of 512
SCALE = 1.0 / np.sqrt(HS)
WSCALE = 64.0  # fp8 hi/lo weight pre-scale (keeps residuals out of subnormals)
ASCALE = 32.0  # fp8 hi/lo pre-scale for the attention output path

_CACHE = {}


def _build():
    import concourse.bass as bass
    import concourse.bacc as bacc
    import concourse.mybir as mybir
    from concourse.tile import TileContext

    f32 = mybir.dt.float32
    f16 = mybir.dt.float16
    f8 = mybir.dt.float8e4
    DR = mybir.MatmulPerfMode.DoubleRow

    nc = bacc.Bacc()
    # x pre-transposed and hi/lo fp8-split host-side:
    # col = s*16384 + ki*2048 + seq, row = dm%128 (ki = dm//128)
    xhl_in = nc.dram_tensor("xhl_in", [128, 2 * 8 * S], f8, kind="ExternalInput")
    woh_in = nc.dram_tensor("woh_in", [128, 4 * 1024], f8, kind="ExternalInput")
    wol_in = nc.dram_tensor("wol_in", [128, 4 * 1024], f8, kind="ExternalInput")
    # fp8 hi/lo Q/K weights, one [qh|ql|kh|kl] block per pair so each
    # pair preload is a single DMA
    wqk_in = nc.dram_tensor("wqk_in", [128, 4 * 4096], f8, kind="ExternalInput")
    wvh_in = nc.dram_tensor("wvh_in", [128, 8 * 512], f8, kind="ExternalInput")
    wvl_in = nc.dram_tensor("wvl_in", [128, 4 * 1024], f8, kind="ExternalInput")
    out_ex = nc.dram_tensor("out_partial", [S, DM], f16, kind="ExternalOutput")

    # band plan: for band k-tile with offset dlt, compute cols [c0, 512)
    BAND = {0: 0, 128: 128, 256: 256, 384: 384}

    with TileContext(nc) as tc:
        with tc.tile_pool(name="persist", bufs=1) as persist, \
             tc.tile_pool(name="prj", bufs=4) as prj, \
             tc.tile_pool(name="attp", bufs=2) as attp, \
             tc.tile_pool(name="osbp", bufs=2) as osbp, \
             tc.tile_pool(name="psp", bufs=2, space="PSUM") as psp:

            qt_sb = persist.tile([128, NPAIR * S], f16)          # 2 MB
            kt_sb = persist.tile([128, NPAIR * S], f16)          # 2 MB
            v_sb = persist.tile([128, 16 * HPC * 65], f16)       # 2.1 MB
            at_sb = persist.tile([128, NPAIR * S], f16)          # 2 MB
            ath_sb = persist.tile([128, NPAIR * S], f8)          # 1 MB
            atl_sb = persist.tile([128, NPAIR * S], f8)          # 1 MB
            woh_sb = persist.tile([128, 4 * 1024], f8)           # 0.5 MB
            wol_sb = persist.tile([128, 4 * 1024], f8)           # 0.5 MB
            wvh_sb = persist.tile([128, 8 * 512], f8)            # 0.5 MB
            wvl_sb = persist.tile([128, 4 * 1024], f8)           # 0.5 MB
            wqk_sb = persist.tile([128, 4 * 4096], f8)           # 2 MB
            # xh/xl fp8: col = s*16384 + ki*2048 + seq
            xhl = persist.tile([128, 2 * 8 * S], f8)             # 4 MB
            xhl_s = xhl.rearrange("p (s r) -> p s r", s=2)
            xhl_a = xhl.rearrange("p (a b) -> p a b", b=S)

            # ---------- startup DMAs, ordered by first consumer ----------
            with nc.named_scope("ph1_load"):
                # few large transfers: HWDGE issue overhead dominates many
                # small ones; order tracks the V-proj ki consumption
                def c0_piece(s, k0, k1):
                    view = (lambda t: t.rearrange("p (s2 k b) -> p s2 k b",
                                                  s2=2, b=S))
                    nc.sync.dma_start(
                        view(xhl)[:, s, k0:k1, 0:512],
                        view(xhl_in)[:, s, k0:k1, 0:512])
                # small lead slices let V-proj pos0 start ~3us sooner;
                # the remainder stays in large transfers
                nc.sync.dma_start(wvh_sb[:, 0:1024], wvh_in[:, 0:1024])
                c0_piece(0, 0, 2)
                c0_piece(1, 0, 2)
                nc.sync.dma_start(wvh_sb[:, 1024:4096], wvh_in[:, 1024:4096])
                c0_piece(0, 2, 4)
                c0_piece(1, 2, 4)
                c0_piece(0, 4, 8)
                c0_piece(1, 4, 8)
                nc.sync.dma_start(wvl_sb[:], wvl_in[:])
                nc.sync.dma_start(wqk_sb[:, 0:4096], wqk_in[:, 0:4096])
                nc.sync.dma_start(wqk_sb[:, 4096:8192], wqk_in[:, 4096:8192])

            # "ones" columns of v_sb at 1/WSCALE: sums come out pre-divided,
            # so the reciprocal yields WSCALE/sum and at_sb holds WSCALE*attn
            # (keeps the fp8 hi/lo split of at_sb out of subnormals)
            for pos in range(16):
                nc.vector.memset(
                    v_sb[:, pos * 520:(pos + 1) * 520]
                    .rearrange("p (h c) -> p h c", c=65)[:, :, 64:65],
                    1.0 / ASCALE)

            # persistent causal 0/1 mask for the diagonal 128x128 tile:
            # umask[kp, j] = 1 where j >= kp.  Applied post-exp as a cheap
            # DVE multiply (keeps Pool's queue out of the exp->PV chain).
            umask = persist.tile([128, 128], f16)
            nc.vector.memset(umask[:], 1.0)
            nc.gpsimd.affine_select(
                out=umask[:], in_=umask[:],
                compare_op=mybir.AluOpType.is_ge,
                fill=0.0, base=0, pattern=[[1, 128]],
                channel_multiplier=-1)

            # ---------- projection emitters ----------
            def vproj8_thunks(pos):
                """fp8 DoubleRow V projection of seq tile pos as thunks."""
                thunks = []
                state = {}

                def alloc():
                    state["psv"] = psp.tile([128, 512], f32, tag="projacc",
                                            name="psv")
                thunks.append(alloc)
                for ki in range(8):
                    def mm(ki=ki):
                        nc.tensor.matmul(
                            state["psv"][:],
                            xhl_s[:, :, ki * 2048 + pos * 128:
                                  ki * 2048 + pos * 128 + 128],
                            wvh_sb[:, ki * 512:(ki + 1) * 512]
                            .rearrange("p (o c) -> p o c", o=1)
                            .broadcast_to([128, 2, 512]),
                            start=(ki == 0), stop=False, perf_mode=DR)
                    thunks.append(mm)
                for kd in range(4):
                    def mm(kd=kd):
                        nc.tensor.matmul(
                            state["psv"][:],
                            xhl_a[:, 2 * kd:2 * kd + 2,
                                  pos * 128:pos * 128 + 128],
                            wvl_sb.rearrange("p (d s c) -> p d s c",
                                             s=2, c=512)[:, kd],
                            start=False, stop=(kd == 3), perf_mode=DR)
                    thunks.append(mm)

                def cp():
                    with nc.allow_low_precision(reason="f16 V"):
                        nc.vector.tensor_scalar_mul(
                            v_sb[:, pos * 520: (pos + 1) * 520]
                            .rearrange("p (h c) -> p h c", c=65)[:, :, 0:64],
                            state["psv"].rearrange("p (h c) -> p h c", c=64),
                            1.0 / WSCALE)
                thunks.append(cp)
                return thunks

            def qkproj_thunks(p, qcs=range(4), state=None, preload=True):
                """fp8 DoubleRow Q/K projection of pair p over q-chunks qcs."""
                thunks = []
                if state is None:
                    state = {}
                if preload:
                    def pre(p=p):
                        wqk = wqk_sb[:, p * 4096:(p + 1) * 4096]
                        state["q"] = (wqk[:, 0:1024], wqk[:, 1024:2048])
                        state["k"] = (wqk[:, 2048:3072], wqk[:, 3072:4096])
                    thunks.append(pre)

                for key, dest in (("q", qt_sb), ("k", kt_sb)):
                    for qc in qcs:
                        for ki in range(8):
                            def mm(key=key, qc=qc, ki=ki):
                                if ki == 0:
                                    state["acc"] = psp.tile(
                                        [128, 512], f32, tag="projacc",
                                        name="projacc")
                                nc.tensor.matmul(
                                    state["acc"][:],
                                    state[key][0][:, ki * 128:(ki + 1) * 128]
                                    .rearrange("p (o m) -> p o m", o=1)
                                    .broadcast_to([128, 2, 128]),
                                    xhl_s[:, :, ki * 2048 + qc * 512:
                                          ki * 2048 + (qc + 1) * 512],
                                    start=(ki == 0), stop=False, perf_mode=DR)
                            thunks.append(mm)
                        for kd in range(4):
                            def mm(key=key, qc=qc, kd=kd):
                                nc.tensor.matmul(
                                    state["acc"][:],
                                    state[key][1].rearrange(
                                        "p (d s m) -> p d s m",
                                        s=2, m=128)[:, kd],
                                    xhl_a[:, 2 * kd:2 * kd + 2,
                                          qc * 512:(qc + 1) * 512],
                                    start=False, stop=(kd == 3), perf_mode=DR)
                            thunks.append(mm)

                        def cp(dest=dest, qc=qc, p=p):
                            with nc.allow_low_precision(reason="f16 qkv"):
                                nc.vector.tensor_scalar_mul(
                                    dest[:, p * S + qc * 512:
                                         p * S + (qc + 1) * 512],
                                    state["acc"][:], 1.0 / WSCALE)
                        thunks.append(cp)
                return thunks

            def outproj_thunks(qt, js=range(4)):
                """Out-projection of q16 tiles qt*4+j for j in js."""
                thunks = []
                state = {}
                for j in js:
                    q16 = qt * 4 + j

                    def alloc(q16=q16):
                        state["pso"] = [
                            psp.tile([128, 512], f32, tag="projacc",
                                     name=f"pso{fc}") for fc in range(2)]
                        state["osb"] = osbp.tile([128, 1024], f16, tag="osb",
                                                 bufs=4, name="osb")
                    thunks.append(alloc)

                    def at2(src, pp, q16):
                        """[128, 2, 128] DoubleRow slots over pairs pp, pp+1."""
                        return src.rearrange("p (e c) -> p e c", e=4) \
                            [:, pp:pp + 2, q16 * 128:(q16 + 1) * 128]

                    for term, src, wsb in (("hh", ath_sb, woh_sb),
                                           ("lh", atl_sb, woh_sb),
                                           ("hl", ath_sb, wol_sb)):
                        for pp in (0, 2):
                            for fc in range(2):
                                def mm(term=term, src=src, wsb=wsb,
                                       pp=pp, fc=fc, q16=q16):
                                    nc.tensor.matmul(
                                        state["pso"][fc][:],
                                        at2(src, pp, q16),
                                        wsb.rearrange(
                                            "p (e c) -> p e c",
                                            e=4)[:, pp:pp + 2,
                                                 fc * 512:(fc + 1) * 512],
                                        start=(term == "hh" and pp == 0),
                                        stop=(term == "hl" and pp == 2),
                                        perf_mode=DR)
                                thunks.append(mm)

                    def cp(q16=q16):
                        for fc in range(2):
                            with nc.allow_low_precision(reason="descale"):
                                nc.vector.tensor_scalar_mul(
                                    state["osb"][:, fc * 512:(fc + 1) * 512],
                                    state["pso"][fc][:],
                                    1.0 / (ASCALE * WSCALE))
                        nc.sync.dma_start(
                            out_ex[q16 * 128:(q16 + 1) * 128, :], state["osb"][:])
                    thunks.append(cp)
                return thunks

            # per-pair stationary-weight slices out of the persistent wqk_sb
            pstates = {}
            for pp_ in range(NPAIR):
                wp_ = wqk_sb[:, pp_ * 4096:(pp_ + 1) * 4096]
                pstates[pp_] = {"q": (wp_[:, 0:1024], wp_[:, 1024:2048]),
                                "k": (wp_[:, 2048:3072], wp_[:, 3072:4096])}

            # ---------- bootstrap: V(0-3) + pair-0 qc0 ----------
            with nc.named_scope("ph2_boot"):
                boot = []
                for pos in range(4):
                    boot += vproj8_thunks(pos)
                # seq chunks 1-3 of xh/xl (strided 3D copies), with the
                # remaining weight blocks interleaved by first consumer
                for c in range(1, 4):
                    for s in range(2):
                        nc.sync.dma_start(
                            xhl_s[:, s].rearrange("p (k b) -> p k b", k=8)
                            [:, :, c * 512:(c + 1) * 512],
                            xhl_in.rearrange("p (s k b) -> p s k b",
                                             s=2, b=S)[:, s, :,
                                                       c * 512:(c + 1) * 512])
                    if c == 1:
                        nc.sync.dma_start(wqk_sb[:, 8192:12288],
                                          wqk_in[:, 8192:12288])
                    elif c == 2:
                        nc.sync.dma_start(wqk_sb[:, 12288:16384],
                                          wqk_in[:, 12288:16384])
                    else:
                        nc.sync.dma_start(woh_sb[:], woh_in[:])
                        nc.sync.dma_start(wol_sb[:], wol_in[:])
                for t in boot:
                    t()
                for t in qkproj_thunks(0, qcs=[0], state=pstates[0],
                                       preload=False):
                    t()

            # persistent PV accumulators: 4 chains x 65 cols per bank tile.
            # The first matmul of each qt (start=True) zeroes the whole psum
            # bank region, so all four chains start from 0 with no memset.
            pvq = [psp.tile([128, 512], f32, tag=f"pvq{i}", name=f"pvq{i}",
                            bufs=1) for i in range(2)]

            # ---------- global filler stream (deadline-paced) ----------
            # blocks run qt-major: b = 4*qt + p.  All projection work forms
            # one FIFO of units; unit deadlines say which block needs them.
            # Units drain into the ACT-surplus of earlier slots (paced) and
            # are force-drained at their deadline block.
            fillerq = []  # [deadline_block, thunks, cursor]

            def push_unit(deadline, thunks):
                if thunks:
                    fillerq.append([deadline, list(thunks), 0])

            def filler_fire(n):
                fired = 0
                while fired < n and fillerq:
                    u = fillerq[0]
                    u[1][u[2]]()
                    u[2] += 1
                    fired += 1
                    if u[2] >= len(u[1]):
                        fillerq.pop(0)
                return fired

            def filler_due(maxdl):
                return sum(len(u[1]) - u[2] for u in fillerq if u[0] <= maxdl)

            def drain_due(b):
                while fillerq and fillerq[0][0] <= b:
                    u = fillerq[0]
                    while u[2] < len(u[1]):
                        u[1][u[2]]()
                        u[2] += 1
                    fillerq.pop(0)

            for rnd in range(NQT):
                for fp in range(1, NPAIR):
                    push_unit(4 * rnd + fp,
                              qkproj_thunks(fp, qcs=[rnd], state=pstates[fp],
                                            preload=False))
                if rnd + 1 < NQT:
                    push_unit(4 * (rnd + 1),
                              qkproj_thunks(0, qcs=[rnd + 1],
                                            state=pstates[0], preload=False))
                    for j in range(4):
                        push_unit(4 * (rnd + 1),
                                  vproj8_thunks(4 * (rnd + 1) + j))

            with nc.named_scope("ph3_attn"):
                from collections import deque
                carry = deque()   # per-slot post-lists left over from the
                                  # previous block (trailing PV + tails)
                rem_slots = sum(4 * (q * 4 + 4) for q in range(NQT))
                for qt in range(NQT):
                  for p in range(NPAIR):
                    b = 4 * qt + p
                    drain_due(b)
                    qs = qt * 512
                    nkt = qs // 128 + 4
                    rc = attp.tile([128, 8], f32, tag="rc", bufs=6,
                                   name="rc")
                    an = attp.tile([128, 512], f16, tag="an", bufs=6,
                                   name="an")
                    ptiles = {}

                    def softmax_tail(i, p=p, qs=qs, rc=rc, an=an):
                        """reciprocal + normalize of pvq[i] (qsubs 2i,
                        2i+1) into the an tile, then transpose out its
                        half of at_sb so consumers start early."""
                        nc.vector.reciprocal(
                            rc.rearrange("p (i g o) -> p i g o",
                                         i=2, o=1)[:, i],
                            pvq[i][:, 0:260]
                            .rearrange("p (g c) -> p g c", c=65)[:, :, 64:65])
                        for qs2 in range(2):
                            qsub = 2 * i + qs2
                            base = i * 4 + qs2 * 2
                            with nc.allow_low_precision(reason="f16 attn"):
                                # both heads in one op: per-(partition,h)
                                # reciprocal enters as a 0-stride bcast
                                nc.vector.tensor_mul(
                                    an[:, qsub * 128:(qsub + 1) * 128]
                                    .rearrange("p (s d) -> p s d", s=2),
                                    pvq[i][:, qs2 * 130: qs2 * 130 + 130]
                                    .rearrange("p (s c) -> p s c",
                                               c=65)[:, :, 0:64],
                                    rc[:, base:base + 2]
                                    .rearrange("p (s o) -> p s o", o=1)
                                    .broadcast_to([128, 2, 64]))
                        cols = slice(p * S + qs + i * 256,
                                     p * S + qs + i * 256 + 256)
                        nc.sync.dma_start(
                            at_sb[:, cols].rearrange("d (e j) -> d e j",
                                                     e=2),
                            an[:, i * 256:(i + 1) * 256], transpose=True)
                        # fp8 hi/lo split for the out-projection (Pool)
                        with nc.allow_low_precision(reason="fp8 at"):
                            nc.gpsimd.tensor_copy(ath_sb[:, cols],
                                                  at_sb[:, cols])
                            nc.gpsimd.tensor_sub(atl_sb[:, cols],
                                                 at_sb[:, cols],
                                                 ath_sb[:, cols])

                    def pv_bank(i, ktm, p=p, qs=qs, ptiles=ptiles):
                        """PV chains of bank i (qsubs 2i, 2i+1) for ktm."""
                        ptile, c0 = ptiles[ktm]
                        for h in range(2):
                            for qsub in range(max(2 * i, c0 // 128),
                                              2 * i + 2):
                                po = (qsub % 2) * 130 + h * 65
                                nc.tensor.matmul(
                                    pvq[i][:, po:po + 65],
                                    ptile[:, h * 512 + qsub * 128:
                                          h * 512 + (qsub + 1) * 128],
                                    v_sb[:, ktm * 520 + (2 * p + h) * 65:
                                         ktm * 520 + (2 * p + h) * 65 + 65],
                                    start=(ktm == 0 and h == 0
                                           and qsub == 2 * i),
                                    stop=(h == 1 and qsub == 2 * i + 1
                                          and ktm == qs // 128 + qsub))

                    def posts_for(kt, nkt=nkt, qt=qt, p=p, ptiles=ptiles,
                                  pv_bank=pv_bank, softmax_tail=softmax_tail):
                        ts = []
                        if 2 <= kt < nkt + 2:   # bank 0: PV for kt-2
                            ktm0 = kt - 2
                            def t0(ktm0=ktm0):
                                if ptiles[ktm0][1] // 128 < 2:
                                    pv_bank(0, ktm0)
                                if ktm0 == nkt - 3:
                                    softmax_tail(0)
                                    if p == NPAIR - 1:
                                        # rows qs..qs+256 of all 4 pairs are
                                        # split: first-half out-proj is ready
                                        push_unit(98, outproj_thunks(
                                            qt, js=[0, 1]))
                            ts.append(t0)
                        if kt >= 4:             # bank 1: PV for kt-4
                            ktm1 = kt - 4
                            def t1(ktm1=ktm1):
                                pv_bank(1, ktm1)
                                ptiles.pop(ktm1)
                            ts.append(t1)
                        if kt == nkt + 3:
                            def t2():
                                softmax_tail(1)
                                if p == NPAIR - 1:
                                    push_unit(98, outproj_thunks(
                                        qt, js=[2, 3]))
                            ts.append(t2)
                        return ts

                    facc = 0.0
                    ffired = 0
                    for kt in range(nkt):
                        dlt = kt * 128 - qs
                        c0 = BAND[dlt] if dlt >= 0 else 0
                        sc = psp.tile([128, 1024], f32, tag="sc",
                                      name="sc")
                        for h in range(2):
                            nc.tensor.matmul(
                                sc[:, h * 512 + c0: (h + 1) * 512],
                                kt_sb[h * 64:(h + 1) * 64,
                                      p * S + kt * 128: p * S + (kt + 1) * 128],
                                qt_sb[h * 64:(h + 1) * 64,
                                      p * S + qs + c0: p * S + qs + 512],
                                start=True, stop=True)
                        ptile = attp.tile([128, 1024], f16, tag="pt",
                                          bufs=8, name="pt")
                        sc_v = sc.rearrange("k (h q) -> k h q", h=2)[:, :, c0:512]
                        pt_v = ptile.rearrange("k (h q) -> k h q", h=2)[:, :, c0:512]
                        with nc.allow_low_precision(reason="f16 probs"):
                            nc.scalar.activation(
                                pt_v, sc_v, mybir.ActivationFunctionType.Exp,
                                bias=0.0, scale=float(SCALE))
                        if dlt >= 0:
                            blk = ptile.rearrange(
                                "k (h q) -> k h q",
                                h=2)[:, :, dlt:dlt + 128]
                            with nc.allow_low_precision(reason="mask"):
                                nc.vector.tensor_mul(
                                    blk, blk,
                                    umask.rearrange("k (o j) -> k o j", o=1)
                                    .broadcast_to([128, 2, 128]))
                        ptiles[kt] = (ptile, c0)
                        # previous block's trailing PV overlaps our scores
                        if carry:
                            for t in carry.popleft():
                                t()
                        for t in posts_for(kt):
                            t()
                        # filler pump: fill the ACT surplus of this slot,
                        # keep up with upcoming deadlines, and never let the
                        # global backlog outpace the remaining slot budget
                        act_ns = 1.667 * (512 - c0) + 185
                        pe_ns = 0.833 * (512 - c0) + 250
                        rate = max(act_ns - pe_ns, 0.0) / 107.0
                        rate = max(rate, filler_due(b + 1) / max(nkt - kt, 1))
                        rate = max(rate, filler_due(99) / max(rem_slots, 1))
                        facc += min(rate, 6.0)
                        ffired += filler_fire(int(facc) - ffired)
                        rem_slots -= 1
                    for kt in range(nkt, nkt + 4):
                        carry.append(posts_for(kt))

            # ---------- phase 4: drain trailing PV + final out-proj ----------
            with nc.named_scope("ph4_outproj"):
                while carry:
                    for t in carry.popleft():
                        t()
                drain_due(999)

    nc.compile()
    return nc


def _get_program():
    if "nc" not in _CACHE:
        _CACHE["nc"] = _build()
    return _CACHE["nc"]


def _split8(w):
    import ml_dtypes
    f8 = ml_dtypes.float8_e4m3
    h = w.astype(f8)
    l = (w - h.astype(np.float32)).astype(f8)
    return h, l


def _make_in_maps(x, w_qkv, w_out):
    import ml_dtypes
    f8np = ml_dtypes.float8_e4m3
    x = np.asarray(x, dtype=np.float32).astype(np.float16)
    w_qkv = np.asarray(w_qkv, dtype=np.float32)
    w_out = np.asarray(w_out, dtype=np.float32)

    w_q, w_k, w_v = w_qkv[:, 0:DM], w_qkv[:, DM:2 * DM], w_qkv[:, 2 * DM:3 * DM]

    def pack_x(xb16):
        """[2048, 1024] fp16 -> [128, 2*8*2048] fp8 hi/lo, transposed."""
        xh = xb16.astype(f8np)
        xl = (xb16.astype(np.float32) - xh.astype(np.float32)).astype(f8np)

        def tr(part):
            return part.T.reshape(8, 128, S).transpose(1, 0, 2).reshape(128, 8 * S)
        return np.ascontiguousarray(
            np.concatenate([tr(xh), tr(xl)], axis=1))

    xhl_packs = [pack_x(x[b]) for b in range(B)]

    def pack_qk(w_core):
        """[1024, 512] -> hi [128, 4*2048], lo [128, 4*1024] DoubleRow packs."""
        w_core = w_core * WSCALE
        his, los = [], []
        for p in range(NPAIR):
            h, l = _split8(w_core[:, p * 128:(p + 1) * 128])
            # hi: [r, ki, m] (DoubleRow dup comes from a 0-stride broadcast)
            hp = h.reshape(8, 128, 128).transpose(1, 0, 2)
            his.append(np.ascontiguousarray(hp.reshape(128, 1024)))
            # lo: [r, kd, s, m] with s the ki sub-chunk
            lp = l.reshape(4, 2, 128, 128).transpose(2, 0, 1, 3)
            los.append(lp.reshape(128, 1024))
        return np.concatenate(his, 1), np.concatenate(los, 1)

    def pack_v(w_core):
        """[1024, 512] -> hi [128, 8*512], lo [128, 4*1024]."""
        h, l = _split8(w_core * WSCALE)
        hp = h.reshape(8, 128, 512).transpose(1, 0, 2)
        lp = l.reshape(4, 2, 128, 512).transpose(2, 0, 1, 3)
        return (np.ascontiguousarray(hp.reshape(128, 4096)),
                np.ascontiguousarray(lp.reshape(128, 4096)))

    in_maps = []
    for c in range(8):
        b, s = c // 2, c % 2
        cols = slice(s * HPC * HS, (s + 1) * HPC * HS)
        wqh, wql = pack_qk(w_q[:, cols])
        wkh, wkl = pack_qk(w_k[:, cols])
        # one [qh|ql|kh|kl] block per pair
        wqk = np.concatenate(
            [np.concatenate([wqh[:, p * 1024:(p + 1) * 1024],
                             wql[:, p * 1024:(p + 1) * 1024],
                             wkh[:, p * 1024:(p + 1) * 1024],
                             wkl[:, p * 1024:(p + 1) * 1024]], axis=1)
             for p in range(NPAIR)], axis=1)
        wvh, wvl = pack_v(w_v[:, cols])
        # wo: [512, 1024] scaled, fp8 hi/lo, [r, pair, c] layout
        woh8, wol8 = _split8(w_out[cols, :] * WSCALE)
        woh = np.ascontiguousarray(
            woh8.reshape(4, 128, 1024).transpose(1, 0, 2).reshape(128, 4096))
        wol = np.ascontiguousarray(
            wol8.reshape(4, 128, 1024).transpose(1, 0, 2).reshape(128, 4096))
        in_maps.append({
            "xhl_in": xhl_packs[b],
            "woh_in": woh, "wol_in": wol,
            "wqk_in": np.ascontiguousarray(wqk),
            "wvh_in": wvh, "wvl_in": wvl,
        })
    return in_maps


def kernel(x: np.ndarray, w_qkv: np.ndarray, w_out: np.ndarray) -> np.ndarray:
    from concourse.bass_utils import run_bass_kernel_spmd

    nc = _get_program()
    in_maps = _make_in_maps(x, w_qkv, w_out)
    res = run_bass_kernel_spmd(nc, in_maps, core_ids=list(range(8)))
    out = np.empty((B, S, DM), dtype=np.float32)
    for b in range(B):
        out[b] = (res.results[2 * b]["out_partial"].astype(np.float32)
                  + res.results[2 * b + 1]["out_partial"].astype(np.float32))
    return out



# revision 24
# speedup vs baseline: 1.1971x; 1.0013x over previous
"""Multi-head causal attention (B=4, S=2048, D=1024, H=16) on 8 Trainium2 cores.

Sharding: core c handles batch c//2 and heads [8*(c%2), 8*(c%2)+8).
Each core computes a partial output (its 8 heads' contribution to all 2048
rows of its batch); the host sums the two partials per batch. No collectives.

Pipeline (fp16 operands, fp8e4m3 hi/lo DoubleRow matmuls, f32 PSUM):
  x    arrives pre-transposed and hi/lo fp8-split from the host (xhl_in);
         weights arrive fp8 hi/lo, pre-scaled by 64 so the lo residuals
         stay out of e4m3 subnormals, packed in DoubleRow slot layout
  Q/K/V projections: DoubleRow fp8 matmuls at 0.5 cycles/row,
         w^T x ~= wh^T(xh+xl) + wl^T xh (relative error ~0.1%); the
         un-duplicated hi weights enter via 0-stride broadcast APs
  scores^T = Kt^T Qt per (pair, 512-q tile, 128-k tile), band-narrowed
  P^T  = exp(scores/8) on ACT, causal diag masked in place by one strided
         gpsimd affine_select per k-tile (both heads at once)
  PV flipped: attn[q,65] += P^T[k,qsub]^T V[k,65] (psum bank = 4 chains
         behind one start=True region-zero; the two banks are software-
         pipelined 2 kt-iterations apart so softmax tails overlap PV)
  column 64 of V holds 1/32 -> per-partition softmax sums -> DVE
         reciprocal + tensor_scalar normalize (at_sb holds 32*attn);
         attn_n -> at_sb via one DMA-crossbar transpose per half-qt,
         then Pool casts the fp8 hi/lo copy used by the out-projection
  out-proj (at^T w_out) in fp8 DoubleRow, interleaved into pair-3's
         attention as filler; partials summed on the host per batch

Q/K projections of pair p+1 are interleaved as filler into the attention
of pair p (fractionally paced); the out projection is the filler of
pair 3. Startup DMAs are few and large (HWDGE issue overhead dominates
small transfers) and ordered by first consumer.
"""
import sys
sys.path.insert(0, '/opt/trn_rl_repo')

import numpy as np

B, S, DM, H, HS = 4, 2048, 1024, 16, 64
HPC = 8      # heads per core
NPAIR = 4    # head pairs per core
NQT = 4      # q tiles of 512
SCALE = 1.0 / np.sqrt(HS)
WSCALE = 64.0  # fp8 hi/lo weight pre-scale (keeps residuals out of subnormals)
ASCALE = 32.0  # fp8 hi/lo pre-scale for the attention output path

_CACHE = {}


def _build():
    import concourse.bass as bass
    import concourse.bacc as bacc
    import concourse.mybir as mybir
    from concourse.tile import TileContext

    f32 = mybir.dt.float32
    f16 = mybir.dt.float16
    f8 = mybir.dt.float8e4
    DR = mybir.MatmulPerfMode.DoubleRow

    nc = bacc.Bacc()
    # x pre-transposed and hi/lo fp8-split host-side:
    # col = s*16384 + ki*2048 + seq, row = dm%128 (ki = dm//128)
    xhl_in = nc.dram_tensor("xhl_in", [128, 2 * 8 * S], f8, kind="ExternalInput")
    woh_in = nc.dram_tensor("woh_in", [128, 4 * 1024], f8, kind="ExternalInput")
    wol_in = nc.dram_tensor("wol_in", [128, 4 * 1024], f8, kind="ExternalInput")
    # fp8 hi/lo Q/K weights, one [qh|ql|kh|kl] block per pair so each
    # pair preload is a single DMA
    wqk_in = nc.dram_tensor("wqk_in", [128, 4 * 4096], f8, kind="ExternalInput")
    wvh_in = nc.dram_tensor("wvh_in", [128, 8 * 512], f8, kind="ExternalInput")
    wvl_in = nc.dram_tensor("wvl_in", [128, 4 * 1024], f8, kind="ExternalInput")
    out_ex = nc.dram_tensor("out_partial", [S, DM], f16, kind="ExternalOutput")

    # band plan: for band k-tile with offset dlt, compute cols [c0, 512)
    BAND = {0: 0, 128: 128, 256: 256, 384: 384}

    with TileContext(nc) as tc:
        with tc.tile_pool(name="persist", bufs=1) as persist, \
             tc.tile_pool(name="prj", bufs=4) as prj, \
             tc.tile_pool(name="attp", bufs=2) as attp, \
             tc.tile_pool(name="osbp", bufs=2) as osbp, \
             tc.tile_pool(name="psp", bufs=2, space="PSUM") as psp:

            qt_sb = persist.tile([128, NPAIR * S], f16)          # 2 MB
            kt_sb = persist.tile([128, NPAIR * S], f16)          # 2 MB
            v_sb = persist.tile([128, 16 * HPC * 65], f16)       # 2.1 MB
            at_sb = persist.tile([128, NPAIR * S], f16)          # 2 MB
            ath_sb = persist.tile([128, NPAIR * S], f8)          # 1 MB
            atl_sb = persist.tile([128, NPAIR * S], f8)          # 1 MB
            woh_sb = persist.tile([128, 4 * 1024], f8)           # 0.5 MB
            wol_sb = persist.tile([128, 4 * 1024], f8)           # 0.5 MB
            wvh_sb = persist.tile([128, 8 * 512], f8)            # 0.5 MB
            wvl_sb = persist.tile([128, 4 * 1024], f8)           # 0.5 MB
            wqk_sb = persist.tile([128, 4 * 4096], f8)           # 2 MB
            # xh/xl fp8: col = s*16384 + ki*2048 + seq
            xhl = persist.tile([128, 2 * 8 * S], f8)             # 4 MB
            xhl_s = xhl.rearrange("p (s r) -> p s r", s=2)
            xhl_a = xhl.rearrange("p (a b) -> p a b", b=S)

            # ---------- startup DMAs, ordered by first consumer ----------
            with nc.named_scope("ph1_load"):
                # few large transfers: HWDGE issue overhead dominates many
                # small ones; order tracks the V-proj ki consumption
                def c0_piece(s, k0, k1):
                    view = (lambda t: t.rearrange("p (s2 k b) -> p s2 k b",
                                                  s2=2, b=S))
                    nc.sync.dma_start(
                        view(xhl)[:, s, k0:k1, 0:512],
                        view(xhl_in)[:, s, k0:k1, 0:512])
                # small lead slices let V-proj pos0 start ~3us sooner;
                # the remainder stays in large transfers
                nc.sync.dma_start(wvh_sb[:, 0:1024], wvh_in[:, 0:1024])
                c0_piece(0, 0, 2)
                c0_piece(1, 0, 2)
                nc.sync.dma_start(wvh_sb[:, 1024:4096], wvh_in[:, 1024:4096])
                c0_piece(0, 2, 4)
                c0_piece(1, 2, 4)
                c0_piece(0, 4, 8)
                c0_piece(1, 4, 8)
                nc.sync.dma_start(wvl_sb[:], wvl_in[:])
                nc.sync.dma_start(wqk_sb[:, 0:4096], wqk_in[:, 0:4096])
                nc.sync.dma_start(wqk_sb[:, 4096:8192], wqk_in[:, 4096:8192])

            # "ones" columns of v_sb at 1/WSCALE: sums come out pre-divided,
            # so the reciprocal yields WSCALE/sum and at_sb holds WSCALE*attn
            # (keeps the fp8 hi/lo split of at_sb out of subnormals)
            for pos in range(16):
                nc.vector.memset(
                    v_sb[:, pos * 520:(pos + 1) * 520]
                    .rearrange("p (h c) -> p h c", c=65)[:, :, 64:65],
                    1.0 / ASCALE)

            # persistent causal 0/1 mask for the diagonal 128x128 tile:
            # umask[kp, j] = 1 where j >= kp.  Applied post-exp as a cheap
            # DVE multiply (keeps Pool's queue out of the exp->PV chain).
            umask = persist.tile([128, 128], f16)
            nc.vector.memset(umask[:], 1.0)
            nc.gpsimd.affine_select(
                out=umask[:], in_=umask[:],
                compare_op=mybir.AluOpType.is_ge,
                fill=0.0, base=0, pattern=[[1, 128]],
                channel_multiplier=-1)

            # ---------- projection emitters ----------
            def vproj8_thunks(pos):
                """fp8 DoubleRow V projection of seq tile pos as thunks."""
                thunks = []
                state = {}

                def alloc():
                    state["psv"] = psp.tile([128, 512], f32, tag="projacc",
                                            name="psv")
                thunks.append(alloc)
                for ki in range(8):
                    def mm(ki=ki):
                        nc.tensor.matmul(
                            state["psv"][:],
                            xhl_s[:, :, ki * 2048 + pos * 128:
                                  ki * 2048 + pos * 128 + 128],
                            wvh_sb[:, ki * 512:(ki + 1) * 512]
                            .rearrange("p (o c) -> p o c", o=1)
                            .broadcast_to([128, 2, 512]),
                            start=(ki == 0), stop=False, perf_mode=DR)
                    thunks.append(mm)
                for kd in range(4):
                    def mm(kd=kd):
                        nc.tensor.matmul(
                            state["psv"][:],
                            xhl_a[:, 2 * kd:2 * kd + 2,
                                  pos * 128:pos * 128 + 128],
                            wvl_sb.rearrange("p (d s c) -> p d s c",
                                             s=2, c=512)[:, kd],
                            start=False, stop=(kd == 3), perf_mode=DR)
                    thunks.append(mm)

                def cp():
                    with nc.allow_low_precision(reason="f16 V"):
                        nc.vector.tensor_scalar_mul(
                            v_sb[:, pos * 520: (pos + 1) * 520]
                            .rearrange("p (h c) -> p h c", c=65)[:, :, 0:64],
                            state["psv"].rearrange("p (h c) -> p h c", c=64),
                            1.0 / WSCALE)
                thunks.append(cp)
                return thunks

            def qkproj_thunks(p, qcs=range(4), state=None, preload=True):
                """fp8 DoubleRow Q/K projection of pair p over q-chunks qcs."""
                thunks = []
                if state is None:
                    state = {}
                if preload:
                    def pre(p=p):
                        wqk = wqk_sb[:, p * 4096:(p + 1) * 4096]
                        state["q"] = (wqk[:, 0:1024], wqk[:, 1024:2048])
                        state["k"] = (wqk[:, 2048:3072], wqk[:, 3072:4096])
                    thunks.append(pre)

                for key, dest in (("q", qt_sb), ("k", kt_sb)):
                    for qc in qcs:
                        for ki in range(8):
                            def mm(key=key, qc=qc, ki=ki):
                                if ki == 0:
                                    state["acc"] = psp.tile(
                                        [128, 512], f32, tag="projacc",
                                        name="projacc")
                                nc.tensor.matmul(
                                    state["acc"][:],
                                    state[key][0][:, ki * 128:(ki + 1) * 128]
                                    .rearrange("p (o m) -> p o m", o=1)
                                    .broadcast_to([128, 2, 128]),
                                    xhl_s[:, :, ki * 2048 + qc * 512:
                                          ki * 2048 + (qc + 1) * 512],
                                    start=(ki == 0), stop=False, perf_mode=DR)
                            thunks.append(mm)
                        for kd in range(4):
                            def mm(key=key, qc=qc, kd=kd):
                                nc.tensor.matmul(
                                    state["acc"][:],
                                    state[key][1].rearrange(
                                        "p (d s m) -> p d s m",
                                        s=2, m=128)[:, kd],
                                    xhl_a[:, 2 * kd:2 * kd + 2,
                                          qc * 512:(qc + 1) * 512],
                                    start=False, stop=(kd == 3), perf_mode=DR)
                            thunks.append(mm)

                        def cp(dest=dest, qc=qc, p=p):
                            with nc.allow_low_precision(reason="f16 qkv"):
                                nc.vector.tensor_scalar_mul(
                                    dest[:, p * S + qc * 512:
                                         p * S + (qc + 1) * 512],
                                    state["acc"][:], 1.0 / WSCALE)
                        thunks.append(cp)
                return thunks

            def outproj_thunks(qt, js=range(4), split_store=False):
                """Out-projection of q16 tiles qt*4+j for j in js."""
                thunks = []
                state = {}
                for j in js:
                    q16 = qt * 4 + j

                    def alloc(q16=q16):
                        state["pso"] = [
                            psp.tile([128, 512], f32, tag="projacc",
                                     name=f"pso{fc}") for fc in range(2)]
                        state["osb"] = osbp.tile([128, 1024], f16, tag="osb",
                                                 bufs=4, name="osb")
                    thunks.append(alloc)

                    def at2(src, pp, q16):
                        """[128, 2, 128] DoubleRow slots over pairs pp, pp+1."""
                        return src.rearrange("p (e c) -> p e c", e=4) \
                            [:, pp:pp + 2, q16 * 128:(q16 + 1) * 128]

                    for term, src, wsb in (("hh", ath_sb, woh_sb),
                                           ("lh", atl_sb, woh_sb),
                                           ("hl", ath_sb, wol_sb)):
                        for pp in (0, 2):
                            for fc in range(2):
                                def mm(term=term, src=src, wsb=wsb,
                                       pp=pp, fc=fc, q16=q16):
                                    nc.tensor.matmul(
                                        state["pso"][fc][:],
                                        at2(src, pp, q16),
                                        wsb.rearrange(
                                            "p (e c) -> p e c",
                                            e=4)[:, pp:pp + 2,
                                                 fc * 512:(fc + 1) * 512],
                                        start=(term == "hh" and pp == 0),
                                        stop=(term == "hl" and pp == 2),
                                        perf_mode=DR)
                                thunks.append(mm)

                    def cp(q16=q16):
                        for fc in range(2):
                            with nc.allow_low_precision(reason="descale"):
                                nc.vector.tensor_scalar_mul(
                                    state["osb"][:, fc * 512:(fc + 1) * 512],
                                    state["pso"][fc][:],
                                    1.0 / (ASCALE * WSCALE))
                            if split_store:
                                nc.sync.dma_start(
                                    out_ex[q16 * 128:(q16 + 1) * 128,
                                           fc * 512:(fc + 1) * 512],
                                    state["osb"][:, fc * 512:(fc + 1) * 512])
                        if not split_store:
                            nc.sync.dma_start(
                                out_ex[q16 * 128:(q16 + 1) * 128, :],
                                state["osb"][:])
                    thunks.append(cp)
                return thunks

            # per-pair stationary-weight slices out of the persistent wqk_sb
            pstates = {}
            for pp_ in range(NPAIR):
                wp_ = wqk_sb[:, pp_ * 4096:(pp_ + 1) * 4096]
                pstates[pp_] = {"q": (wp_[:, 0:1024], wp_[:, 1024:2048]),
                                "k": (wp_[:, 2048:3072], wp_[:, 3072:4096])}

            # ---------- bootstrap: V(0-3) + pair-0 qc0 ----------
            with nc.named_scope("ph2_boot"):
                boot = []
                for pos in range(4):
                    boot += vproj8_thunks(pos)
                # seq chunks 1-3 of xh/xl (strided 3D copies), with the
                # remaining weight blocks interleaved by first consumer
                for c in range(1, 4):
                    for s in range(2):
                        nc.sync.dma_start(
                            xhl_s[:, s].rearrange("p (k b) -> p k b", k=8)
                            [:, :, c * 512:(c + 1) * 512],
                            xhl_in.rearrange("p (s k b) -> p s k b",
                                             s=2, b=S)[:, s, :,
                                                       c * 512:(c + 1) * 512])
                    if c == 1:
                        nc.sync.dma_start(wqk_sb[:, 8192:12288],
                                          wqk_in[:, 8192:12288])
                    elif c == 2:
                        nc.sync.dma_start(wqk_sb[:, 12288:16384],
                                          wqk_in[:, 12288:16384])
                    else:
                        nc.sync.dma_start(woh_sb[:], woh_in[:])
                        nc.sync.dma_start(wol_sb[:], wol_in[:])
                for t in boot:
                    t()
                for t in qkproj_thunks(0, qcs=[0], state=pstates[0],
                                       preload=False):
                    t()

            # persistent PV accumulators: 4 chains x 65 cols per bank tile.
            # The first matmul of each qt (start=True) zeroes the whole psum
            # bank region, so all four chains start from 0 with no memset.
            pvq = [psp.tile([128, 512], f32, tag=f"pvq{i}", name=f"pvq{i}",
                            bufs=1) for i in range(2)]

            # ---------- global filler stream (deadline-paced) ----------
            # blocks run qt-major: b = 4*qt + p.  All projection work forms
            # one FIFO of units; unit deadlines say which block needs them.
            # Units drain into the ACT-surplus of earlier slots (paced) and
            # are force-drained at their deadline block.
            fillerq = []  # [deadline_block, thunks, cursor]

            def push_unit(deadline, thunks):
                if thunks:
                    fillerq.append([deadline, list(thunks), 0])

            def filler_fire(n):
                fired = 0
                while fired < n and fillerq:
                    u = fillerq[0]
                    u[1][u[2]]()
                    u[2] += 1
                    fired += 1
                    if u[2] >= len(u[1]):
                        fillerq.pop(0)
                return fired

            def filler_due(maxdl):
                return sum(len(u[1]) - u[2] for u in fillerq if u[0] <= maxdl)

            def drain_due(b):
                while fillerq and fillerq[0][0] <= b:
                    u = fillerq[0]
                    while u[2] < len(u[1]):
                        u[1][u[2]]()
                        u[2] += 1
                    fillerq.pop(0)

            for rnd in range(NQT):
                for fp in range(1, NPAIR):
                    push_unit(4 * rnd + fp,
                              qkproj_thunks(fp, qcs=[rnd], state=pstates[fp],
                                            preload=False))
                if rnd + 1 < NQT:
                    push_unit(4 * (rnd + 1),
                              qkproj_thunks(0, qcs=[rnd + 1],
                                            state=pstates[0], preload=False))
                    for j in range(4):
                        push_unit(4 * (rnd + 1),
                                  vproj8_thunks(4 * (rnd + 1) + j))

            with nc.named_scope("ph3_attn"):
                from collections import deque
                carry = deque()   # per-slot post-lists left over from the
                                  # previous block (trailing PV + tails)
                rem_slots = sum(4 * (q * 4 + 4) for q in range(NQT))
                for qt in range(NQT):
                  for p in range(NPAIR):
                    b = 4 * qt + p
                    drain_due(b)
                    qs = qt * 512
                    nkt = qs // 128 + 4
                    rc = attp.tile([128, 8], f32, tag="rc", bufs=6,
                                   name="rc")
                    an = attp.tile([128, 512], f16, tag="an", bufs=6,
                                   name="an")
                    ptiles = {}

                    def softmax_tail(i, p=p, qs=qs, rc=rc, an=an,
                                     only_qsub=None):
                        """reciprocal + normalize of pvq[i] (qsubs 2i,
                        2i+1) into the an tile, then transpose out its
                        half of at_sb so consumers start early.  With
                        only_qsub, handle a single 128-row qsub (used to
                        fine-pipeline the very last block's drain)."""
                        qs2s = range(2) if only_qsub is None \
                            else [only_qsub - 2 * i]
                        if only_qsub is None or only_qsub == 2 * i:
                            nc.vector.reciprocal(
                                rc.rearrange("p (i g o) -> p i g o",
                                             i=2, o=1)[:, i],
                                pvq[i][:, 0:260]
                                .rearrange("p (g c) -> p g c",
                                           c=65)[:, :, 64:65])
                        for qs2 in qs2s:
                            qsub = 2 * i + qs2
                            base = i * 4 + qs2 * 2
                            with nc.allow_low_precision(reason="f16 attn"):
                                # both heads in one op: per-(partition,h)
                                # reciprocal enters as a 0-stride bcast
                                nc.vector.tensor_mul(
                                    an[:, qsub * 128:(qsub + 1) * 128]
                                    .rearrange("p (s d) -> p s d", s=2),
                                    pvq[i][:, qs2 * 130: qs2 * 130 + 130]
                                    .rearrange("p (s c) -> p s c",
                                               c=65)[:, :, 0:64],
                                    rc[:, base:base + 2]
                                    .rearrange("p (s o) -> p s o", o=1)
                                    .broadcast_to([128, 2, 64]))
                        if only_qsub is None:
                            cols = slice(p * S + qs + i * 256,
                                         p * S + qs + i * 256 + 256)
                            anv = an[:, i * 256:(i + 1) * 256]
                            atv = at_sb[:, cols].rearrange(
                                "d (e j) -> d e j", e=2)
                        else:
                            cols = slice(p * S + qs + only_qsub * 128,
                                         p * S + qs + only_qsub * 128 + 128)
                            anv = an[:, only_qsub * 128:
                                     (only_qsub + 1) * 128]
                            atv = at_sb[:, cols]
                        nc.sync.dma_start(atv, anv, transpose=True)
                        # fp8 hi/lo split for the out-projection (Pool)
                        with nc.allow_low_precision(reason="fp8 at"):
                            nc.gpsimd.tensor_copy(ath_sb[:, cols],
                                                  at_sb[:, cols])
                            nc.gpsimd.tensor_sub(atl_sb[:, cols],
                                                 at_sb[:, cols],
                                                 ath_sb[:, cols])

                    def pv_bank(i, ktm, p=p, qs=qs, ptiles=ptiles):
                        """PV chains of bank i (qsubs 2i, 2i+1) for ktm."""
                        ptile, c0 = ptiles[ktm]
                        for h in range(2):
                            for qsub in range(max(2 * i, c0 // 128),
                                              2 * i + 2):
                                po = (qsub % 2) * 130 + h * 65
                                nc.tensor.matmul(
                                    pvq[i][:, po:po + 65],
                                    ptile[:, h * 512 + qsub * 128:
                                          h * 512 + (qsub + 1) * 128],
                                    v_sb[:, ktm * 520 + (2 * p + h) * 65:
                                         ktm * 520 + (2 * p + h) * 65 + 65],
                                    start=(ktm == 0 and h == 0
                                           and qsub == 2 * i),
                                    stop=(h == 1 and qsub == 2 * i + 1
                                          and ktm == qs // 128 + qsub))

                    last_blk = (qt == NQT - 1 and p == NPAIR - 1)

                    def posts_for(kt, nkt=nkt, qt=qt, p=p, ptiles=ptiles,
                                  pv_bank=pv_bank, softmax_tail=softmax_tail,
                                  last_blk=last_blk):
                        ts = []
                        if 3 <= kt < nkt + 3:   # bank 0: PV for kt-3
                            ktm0 = kt - 3
                            def t0(ktm0=ktm0):
                                if ptiles[ktm0][1] // 128 < 2:
                                    pv_bank(0, ktm0)
                                if ktm0 == nkt - 3:
                                    softmax_tail(0)
                                    if p == NPAIR - 1:
                                        # rows qs..qs+256 of all 4 pairs are
                                        # split: first-half out-proj is ready
                                        push_unit(98, outproj_thunks(
                                            qt, js=[0, 1]))
                            ts.append(t0)
                        if kt >= 4:             # bank 1: PV for kt-4
                            ktm1 = kt - 4
                            def t1(ktm1=ktm1):
                                pv_bank(1, ktm1)
                                ptiles.pop(ktm1)
                            ts.append(t1)
                        if last_blk:
                            # fine-pipeline the final drain: each qsub's
                            # normalize/transpose/split releases its q16
                            # out-proj immediately
                            if kt == nkt + 2:
                                def t2a():
                                    softmax_tail(1, only_qsub=2)
                                    push_unit(98, outproj_thunks(qt, js=[2]))
                                ts.append(t2a)
                            elif kt == nkt + 3:
                                def t2b():
                                    softmax_tail(1, only_qsub=3)
                                    push_unit(98, outproj_thunks(
                                        qt, js=[3], split_store=True))
                                ts.append(t2b)
                        elif kt == nkt + 3:
                            def t2():
                                softmax_tail(1)
                                if p == NPAIR - 1:
                                    push_unit(98, outproj_thunks(
                                        qt, js=[2, 3]))
                            ts.append(t2)
                        return ts

                    facc = 0.0
                    ffired = 0
                    for kt in range(nkt):
                        dlt = kt * 128 - qs
                        c0 = BAND[dlt] if dlt >= 0 else 0
                        sc = psp.tile([128, 1024], f32, tag="sc",
                                      name="sc")
                        for h in range(2):
                            nc.tensor.matmul(
                                sc[:, h * 512 + c0: (h + 1) * 512],
                                kt_sb[h * 64:(h + 1) * 64,
                                      p * S + kt * 128: p * S + (kt + 1) * 128],
                                qt_sb[h * 64:(h + 1) * 64,
                                      p * S + qs + c0: p * S + qs + 512],
                                start=True, stop=True)
                        ptile = attp.tile([128, 1024], f16, tag="pt",
                                          bufs=8, name="pt")
                        sc_v = sc.rearrange("k (h q) -> k h q", h=2)[:, :, c0:512]
                        pt_v = ptile.rearrange("k (h q) -> k h q", h=2)[:, :, c0:512]
                        with nc.allow_low_precision(reason="f16 probs"):
                            nc.scalar.activation(
                                pt_v, sc_v, mybir.ActivationFunctionType.Exp,
                                bias=0.0, scale=float(SCALE))
                        if dlt >= 0:
                            blk = ptile.rearrange(
                                "k (h q) -> k h q",
                                h=2)[:, :, dlt:dlt + 128]
                            with nc.allow_low_precision(reason="mask"):
                                nc.vector.tensor_mul(
                                    blk, blk,
                                    umask.rearrange("k (o j) -> k o j", o=1)
                                    .broadcast_to([128, 2, 128]))
                        ptiles[kt] = (ptile, c0)
                        # previous block's trailing PV overlaps our scores
                        if carry:
                            for t in carry.popleft():
                                t()
                        for t in posts_for(kt):
                            t()
                        # filler pump: fill the ACT surplus of this slot,
                        # keep up with upcoming deadlines, and never let the
                        # global backlog outpace the remaining slot budget
                        act_ns = 1.667 * (512 - c0) + 185
                        pe_ns = 0.833 * (512 - c0) + 250
                        rate = max(act_ns - pe_ns, 0.0) / 107.0
                        rate = max(rate, filler_due(b + 1) / max(nkt - kt, 1))
                        rate = max(rate, filler_due(99) / max(rem_slots, 1))
                        facc += min(rate, 6.0)
                        ffired += filler_fire(int(facc) - ffired)
                        rem_slots -= 1
                    for kt in range(nkt, nkt + 4):
                        carry.append(posts_for(kt))

            # ---------- phase 4: drain trailing PV + final out-proj ----------
            with nc.named_scope("ph4_outproj"):
                while carry:
                    for t in carry.popleft():
                        t()
                    drain_due(999)
                drain_due(999)

    nc.compile()
    return nc


def _get_program():
    if "nc" not in _CACHE:
        _CACHE["nc"] = _build()
    return _CACHE["nc"]


def _split8(w):
    import ml_dtypes
    f8 = ml_dtypes.float8_e4m3
    h = w.astype(f8)
    l = (w - h.astype(np.float32)).astype(f8)
    return h, l


def _make_in_maps(x, w_qkv, w_out):
    import ml_dtypes
    f8np = ml_dtypes.float8_e4m3
    x = np.asarray(x, dtype=np.float32).astype(np.float16)
    w_qkv = np.asarray(w_qkv, dtype=np.float32)
    w_out = np.asarray(w_out, dtype=np.float32)

    w_q, w_k, w_v = w_qkv[:, 0:DM], w_qkv[:, DM:2 * DM], w_qkv[:, 2 * DM:3 * DM]

    def pack_x(xb16):
        """[2048, 1024] fp16 -> [128, 2*8*2048] fp8 hi/lo, transposed."""
        xh = xb16.astype(f8np)
        xl = (xb16.astype(np.float32) - xh.astype(np.float32)).astype(f8np)

        def tr(part):
            return part.T.reshape(8, 128, S).transpose(1, 0, 2).reshape(128, 8 * S)
        return np.ascontiguousarray(
            np.concatenate([tr(xh), tr(xl)], axis=1))

    xhl_packs = [pack_x(x[b]) for b in range(B)]

    def pack_qk(w_core):
        """[1024, 512] -> hi [128, 4*2048], lo [128, 4*1024] DoubleRow packs."""
        w_core = w_core * WSCALE
        his, los = [], []
        for p in range(NPAIR):
            h, l = _split8(w_core[:, p * 128:(p + 1) * 128])
            # hi: [r, ki, m] (DoubleRow dup comes from a 0-stride broadcast)
            hp = h.reshape(8, 128, 128).transpose(1, 0, 2)
            his.append(np.ascontiguousarray(hp.reshape(128, 1024)))
            # lo: [r, kd, s, m] with s the ki sub-chunk
            lp = l.reshape(4, 2, 128, 128).transpose(2, 0, 1, 3)
            los.append(lp.reshape(128, 1024))
        return np.concatenate(his, 1), np.concatenate(los, 1)

    def pack_v(w_core):
        """[1024, 512] -> hi [128, 8*512], lo [128, 4*1024]."""
        h, l = _split8(w_core * WSCALE)
        hp = h.reshape(8, 128, 512).transpose(1, 0, 2)
        lp = l.reshape(4, 2, 128, 512).transpose(2, 0, 1, 3)
        return (np.ascontiguousarray(hp.reshape(128, 4096)),
                np.ascontiguousarray(lp.reshape(128, 4096)))

    in_maps = []
    for c in range(8):
        b, s = c // 2, c % 2
        cols = slice(s * HPC * HS, (s + 1) * HPC * HS)
        wqh, wql = pack_qk(w_q[:, cols])
        wkh, wkl = pack_qk(w_k[:, cols])
        # one [qh|ql|kh|kl] block per pair
        wqk = np.concatenate(
            [np.concatenate([wqh[:, p * 1024:(p + 1) * 1024],
                             wql[:, p * 1024:(p + 1) * 1024],
                             wkh[:, p * 1024:(p + 1) * 1024],
                             wkl[:, p * 1024:(p + 1) * 1024]], axis=1)
             for p in range(NPAIR)], axis=1)
        wvh, wvl = pack_v(w_v[:, cols])
        # wo: [512, 1024] scaled, fp8 hi/lo, [r, pair, c] layout
        woh8, wol8 = _split8(w_out[cols, :] * WSCALE)
        woh = np.ascontiguousarray(
            woh8.reshape(4, 128, 1024).transpose(1, 0, 2).reshape(128, 4096))
        wol = np.ascontiguousarray(
            wol8.reshape(4, 128, 1024).transpose(1, 0, 2).reshape(128, 4096))
        in_maps.append({
            "xhl_in": xhl_packs[b],
            "woh_in": woh, "wol_in": wol,
            "wqk_in": np.ascontiguousarray(wqk),
            "wvh_in": wvh, "wvl_in": wvl,
        })
    return in_maps


def kernel(x: np.ndarray, w_qkv: np.ndarray, w_out: np.ndarray) -> np.ndarray:
    from concourse.bass_utils import run_bass_kernel_spmd

    nc = _get_program()
    in_maps = _make_in_maps(x, w_qkv, w_out)
    res = run_bass_kernel_spmd(nc, in_maps, core_ids=list(range(8)))
    out = np.empty((B, S, DM), dtype=np.float32)
    for b in range(B):
        out[b] = (res.results[2 * b]["out_partial"].astype(np.float32)
                  + res.results[2 * b + 1]["out_partial"].astype(np.float32))
    return out

